# revision 1
# baseline (speedup 1.0000x reference)
"""DependencyBertSelfAttention Trainium2 kernel (v3).

Sharding: batch B=8 -> one batch element per NeuronCore (8 cores, SPMD).
Per core (full T=1024, C=768, H=12 heads, D=64):

  All projection inputs (x^T, Wq/Wk/Wv) are loaded as bf16 (halves the
  serialized DMA-transfer time); the 1/sqrt(D) score scale is folded into
  Wq on the host. DMAs are issued on the sync queue in critical-path
  order as batched half-tensor transfers.

  PSUM budget (8 banks): scores ring "psS" [128,1024] x2 = 4 banks;
  stage-A scratch "psA" [128,512] x1 = 1 bank; PV accumulators 3 banks
  ([osa|dsa] x 65 cols for t-blocks {0-2}, {3-5}, {6-7} packed 390/390/260).

  Stage A: qk(0) projects through the scores ring before head 0; the
  remaining projections are emitted as 512-column feeder chunks through
  the 1-bank psA scratch, popped one per s-block during heads 0-4, so PE
  fills its idle slots (keeping its p-state ramped) without ever blocking
  the ACT-paced scores pipeline. Q^T/K^T evictions (+bias, bf16) and
  V-chunk bias-adds into the interleaved v_aug layout run on GPSIMD.

  Stage B per head: scores S^T[s,t] = K_h^T.T @ Q_h^T (bf16, k=64);
  branch 1: exp(S) -> pO (bf16); branch 2: sd = S*depT (DVE, bf16 sbuf),
  exp -> pD (bf16), skewed one s-block behind. PV accumulates out[t, 65]
  per t-block over s-blocks (65th column of ones = softmax denominator).
  Normalization by reciprocal(denominator) on GPSIMD into od[tb] =
  [osa | dsa] (bf16).

  Stage C per t-block, software-pipelined: tanh over [osa|dsa], gate
  z = sum(tanh(od)*Wg) via scalar_tensor_tensor accumulate,
  g = 1/(1+exp(-(z+bg))), out = g*osa + (1-g)*dsa, DMA out.

No inter-core communication: each core's batch element is independent.
"""
import sys

sys.path.insert(0, "/opt/trn_rl_repo")

import numpy as np
import ml_dtypes
from contextlib import ExitStack

import concourse.bass as bass
import concourse.tile as tile
from concourse import bacc, mybir

B, T, C, H, D = 8, 1024, 768, 12, 64
CB = C // 128   # 6 channel partition-blocks
SB = T // 128   # 8 s/t blocks
NCORES = 8

F32 = mybir.dt.float32
BF16 = mybir.dt.bfloat16
AF = mybir.ActivationFunctionType
ALU = mybir.AluOpType

# PV psum packing: tile index and column base for each t-block
PV_TILES = ((0, 3), (1, 3), (2, 2))          # (tile idx, n t-blocks)
PV_OF = {}
for _ti, (_i, _n) in enumerate(PV_TILES):
    for _k in range(_n):
        PV_OF[sum(n for _, n in PV_TILES[:_ti]) + _k] = (_ti, _k * 130)


def build_nc(debug=False, repeat=1):
    nc = bacc.Bacc("TRN2", target_bir_lowering=False, debug=False,
                   num_devices=NCORES)

    xT_d = nc.dram_tensor("xT", [C, T], BF16, kind="ExternalInput").ap()
    wq_d = nc.dram_tensor("wq", [C, C], BF16, kind="ExternalInput").ap()
    wk_d = nc.dram_tensor("wk", [C, C], BF16, kind="ExternalInput").ap()
    wv_d = nc.dram_tensor("wv", [C, C], BF16, kind="ExternalInput").ap()
    bqs_d = nc.dram_tensor("bqs", [C], F32, kind="ExternalInput").ap()
    bk_d = nc.dram_tensor("bk", [C], F32, kind="ExternalInput").ap()
    bv_d = nc.dram_tensor("bv", [C], F32, kind="ExternalInput").ap()
    dep_d = nc.dram_tensor("dep", [T, T], BF16, kind="ExternalInput").ap()
    wg_d = nc.dram_tensor("wg", [2 * C], BF16, kind="ExternalInput").ap()
    nbg_d = nc.dram_tensor("nbg", [1], F32, kind="ExternalInput").ap()
    out_d = nc.dram_tensor("out", [T, C], BF16, kind="ExternalOutput").ap()

    def bcast(src_ap, n_free):
        return bass.AP(tensor=src_ap.tensor, offset=src_ap.offset,
                       ap=[[0, 128], [1, n_free]])

    def chunked(src_ap, cb0, ncb, width):
        # dram [C, width] rows cb0*128.. viewed as [p, cb, w] for one DMA
        return bass.AP(tensor=src_ap.tensor, offset=src_ap.offset + cb0 * 128 * width,
                       ap=[[width, 128], [128 * width, ncb], [1, width]])

    def col_chunk(src_ap, cb):
        # dram [C, C] columns cb*128..: [p, kb, j] = src[kb*128+p, cb*128+j]
        return bass.AP(tensor=src_ap.tensor, offset=src_ap.offset + cb * 128,
                       ap=[[C, 128], [128 * C, CB], [1, 128]])

    def col_fold(src_ap, ncb):
        # dram [C] -> [128, ncb]: partition p, col cb = src[cb*128 + p]
        return bass.AP(tensor=src_ap.tensor, offset=src_ap.offset,
                       ap=[[1, 128], [128, ncb]])

    with tile.TileContext(nc, pool_alloc_mode="queue") as tc, ExitStack() as ctx:
      persist = ctx.enter_context(tc.tile_pool(name="persist", bufs=1))
      psS = ctx.enter_context(tc.tile_pool(name="psS", bufs=2, space="PSUM"))
      psA = ctx.enter_context(tc.tile_pool(name="psA", bufs=1, space="PSUM"))
      psPV = ctx.enter_context(tc.tile_pool(name="psPV", bufs=1, space="PSUM"))
      for _rep in range(repeat):
        # ---- persistent tiles
        xTa = persist.tile([128, CB * T], BF16, tag="xTa", name="xTa")
        # weights as per-output-cb column tiles: wqc[cb][p, kb*128+j] =
        # Wq.T[kb*128+p, cb*128+j] — head 0 needs only cb=0 (196KB, not 1.2MB)
        wqc = [persist.tile([128, CB * 128], BF16, tag=f"wqc{i}", name=f"wqc{i}")
               for i in range(CB)]
        wkc = [persist.tile([128, CB * 128], BF16, tag=f"wkc{i}", name=f"wkc{i}")
               for i in range(CB)]
        wva = persist.tile([128, CB * C], BF16, tag="wva", name="wva")
        xT = [xTa[:, i * T:(i + 1) * T] for i in range(CB)]
        wvs = [wva[:, i * C:(i + 1) * C] for i in range(CB)]
        qT = [persist.tile([128, T], BF16, tag=f"qT{i}", name=f"qT{i}") for i in range(CB)]
        kT = [persist.tile([128, T], BF16, tag=f"kT{i}", name=f"kT{i}") for i in range(CB)]
        vaug = [persist.tile([128, H * 65], BF16, tag=f"vaug{i}", name=f"vaug{i}") for i in range(SB)]
        dep_t = [persist.tile([128, T], BF16, tag=f"dep{i}", name=f"dep{i}") for i in range(SB)]
        od = [persist.tile([128, 2 * C], BF16, tag=f"od{i}", name=f"od{i}") for i in range(SB)]
        bqk = persist.tile([128, 2 * CB], F32, tag="bqk", name="bqk")
        bvb = persist.tile([128, C], F32, tag="bvb", name="bvb")
        wgb = persist.tile([128, 2 * C], BF16, tag="wgb", name="wgb")
        nbg_t = persist.tile([128, 1], F32, tag="nbg", name="nbg")

        # warm the ACT exp/tanh table set at t=0 so the first real exp
        # doesn't pay the ~2.7us table load mid-ramp; keep PE continuously
        # busy with dummy matmuls so its p-state ramps to full clock before
        # the first projection matmuls arrive (~6us in)
        with tc.high_priority():
            warm = persist.tile([128, 1], F32, tag="warm", name="warm")
            nc.vector.memset(warm[:], 0.0)
            nc.scalar.activation(warm[:], warm[:], AF.Exp)
            zz = persist.tile([128, 512], BF16, tag="zz", name="zz")
            nc.vector.memset(zz[:], 0.0)
            wz = persist.tile([128, 1], BF16, tag="wz", name="wz")
            nc.vector.memset(wz[:], 0.0)
            for _ in range(14):
                pwarm = psS.tile([128, T], F32, tag="psS", name="pwarm")
                nc.tensor.matmul(pwarm[0:1, 0:512], wz[:], zz[:],
                                 start=True, stop=True)

        # ---- DMAs on the sync queue: the serialized DMA engines process
        # transfers in exactly this order, so startup-critical data (x, the
        # cb=0 weight columns) comes first.
        hc = CB // 2
        def wcol(dst, src, cb):
            nc.sync.dma_start(
                dst[cb][:].rearrange("p (kb j) -> p kb j", j=128),
                col_chunk(src, cb))
        nc.sync.dma_start(
            xTa[:, 0:2 * T].rearrange("p (cb t) -> p cb t", t=T),
            chunked(xT_d, 0, 2, T))
        wcol(wqc, wq_d, 0)
        wcol(wkc, wk_d, 0)
        nc.sync.dma_start(
            xTa[:, 2 * T:4 * T].rearrange("p (cb t) -> p cb t", t=T),
            chunked(xT_d, 2, 2, T))
        nc.sync.dma_start(
            xTa[:, 4 * T:CB * T].rearrange("p (cb t) -> p cb t", t=T),
            chunked(xT_d, 4, 2, T))
        nc.sync.dma_start(bqk[:, 0:CB], col_fold(bqs_d, CB))
        nc.sync.dma_start(bqk[:, CB:2 * CB], col_fold(bk_d, CB))
        nc.sync.dma_start(dep_t[0][:], dep_d[0:128, :])
        nc.sync.dma_start(wva[:].rearrange("p (cb c) -> p cb c", c=C),
                          chunked(wv_d, 0, CB, C))
        nc.sync.dma_start(bvb[:], bcast(bv_d, C))
        nc.sync.dma_start(dep_t[1][:], dep_d[128:256, :])
        for cb in range(1, CB):
            wcol(wqc, wq_d, cb)
            wcol(wkc, wk_d, cb)
            nc.sync.dma_start(dep_t[cb + 1][:],
                              dep_d[(cb + 1) * 128:(cb + 2) * 128, :])
        nc.sync.dma_start(dep_t[7][:], dep_d[7 * 128:8 * 128, :])
        nc.sync.dma_start(wgb[:], bcast(wg_d, 2 * C))
        nc.sync.dma_start(nbg_t[:], bcast(nbg_d, 1))

        # ---- stage-A emission units
        def proj_one(dst, w, cb, bcol, eng):
            # full Q^T or K^T chunk through the scores ring (used for qk(0)
            # ahead of head 0 only): kb-outer so matmuls start as DMA chunks
            # land; per-half evictions unblock the first scores early.
            ps = psS.tile([128, T], F32, tag="psS", name="psS")
            for kb in range(CB):
                for tch in range(2):
                    nc.tensor.matmul(
                        ps[:, tch * 512:(tch + 1) * 512],
                        w[cb][:, kb * 128:(kb + 1) * 128],
                        xT[kb][:, tch * 512:(tch + 1) * 512],
                        start=(kb == 0), stop=(kb == CB - 1))
            for tch in range(2):
                eng.tensor_scalar(
                    dst[cb][:, tch * 512:(tch + 1) * 512],
                    ps[:, tch * 512:(tch + 1) * 512], 1.0,
                    bqk[:, bcol:bcol + 1], ALU.mult, ALU.add)

        def qk_half(dst, w, cb, bcol, tch):
            # 512-col half of a Q^T/K^T chunk through the 1-bank psA scratch
            ps = psA.tile([128, 512], F32, tag="psA", name="psA")
            for kb in range(CB):
                nc.tensor.matmul(
                    ps[:], w[cb][:, kb * 128:(kb + 1) * 128],
                    xT[kb][:, tch * 512:(tch + 1) * 512],
                    start=(kb == 0), stop=(kb == CB - 1))
            nc.vector.tensor_scalar(
                dst[cb][:, tch * 512:(tch + 1) * 512], ps[:], 1.0,
                bqk[:, bcol:bcol + 1], ALU.mult, ALU.add)

        def v_half(sb, half):
            # V natural [s part, c free] 512/256-col chunk -> v_aug
            n0, nw = (0, 512) if half == 0 else (512, 256)
            ps = psA.tile([128, 512], F32, tag="psA", name="psA_v")
            for kb in range(CB):
                nc.tensor.matmul(
                    ps[:, 0:nw],
                    xT[kb][:, sb * 128:(sb + 1) * 128],
                    wvs[kb][:, n0:n0 + nw],
                    start=(kb == 0), stop=(kb == CB - 1))
            va3 = vaug[sb][:].rearrange("p (h d) -> p h d", d=65)
            h0 = n0 // 64
            ps3 = ps[:, 0:nw].rearrange("p (h d) -> p h d", d=64)
            bv3 = bvb[:, n0:n0 + nw].rearrange("p (h d) -> p h d", d=64)
            nc.vector.tensor_add(va3[:, h0:h0 + nw // 64, 0:64], ps3, bv3)
            if half == 0:
                nc.vector.memset(va3[:, :, 64:65], 1.0)

        # Stage-A feeders are emitted BEFORE their consumers (emission order
        # is semantic order in Tile: a reader emitted before its writer reads
        # garbage), but with the priority counter pushed past every normal
        # instruction, so the scheduler only runs them in engine idle slots.
        from contextlib import contextmanager
        feeder_prio = [1 << 20]

        @contextmanager
        def low_priority():
            orig = tc.cur_priority
            tc.cur_priority = feeder_prio[0]
            try:
                yield
            finally:
                feeder_prio[0] = tc.cur_priority
                tc.cur_priority = orig

        def qk_feed(cb, qk, t):
            return lambda: qk_half((qT, kT)[qk], (wqc, wkc)[qk],
                                   cb, qk * CB + cb, t)
        def v_feed(sb, hf):
            return lambda: v_half(sb, hf)
        # batches[h] is emitted BEFORE head h; consumers: vaug half-0 by
        # PV(h0) (during head 0), half-1 by head 8; qk(cb) by head 2*cb.
        batches = [
            [v_feed(sb, 0) for sb in range(SB)],
            [qk_feed(1, qk, t) for qk in range(2) for t in range(2)],
            ([v_feed(sb, 1) for sb in range(SB)]
             + [qk_feed(2, qk, t) for qk in range(2) for t in range(2)]),
            [qk_feed(3, qk, t) for qk in range(2) for t in range(2)],
            [qk_feed(4, qk, t) for qk in range(2) for t in range(2)],
            [qk_feed(5, qk, t) for qk in range(2) for t in range(2)],
        ]

        pb = ctx.enter_context(tc.tile_pool(name="pP", bufs=7))
        sdp = ctx.enter_context(tc.tile_pool(name="sdp", bufs=3))
        rp = ctx.enter_context(tc.tile_pool(name="rp", bufs=16))
        cp = ctx.enter_context(tc.tile_pool(name="stageC", bufs=2))
        zp = ctx.enter_context(tc.tile_pool(name="zP", bufs=3))

        def head(h):
            hb, hoff = h // 2, (h % 2) * 64
            ppvs = []
            pOs, pDs = [], []
            sds = []

            def emit_dsa_pv(psb):
                pD = pb.tile([128, T], BF16, tag="pD", name="pD", bufs=5)
                nc.scalar.activation(pD[:], sds[psb][:], AF.Exp)
                pDs.append(pD)
                for tb in range(SB):
                    ti, base = PV_OF[tb]
                    nc.tensor.matmul(
                        ppvs[ti][:, base + 65:base + 130],
                        pD[:, tb * 128:(tb + 1) * 128],
                        vaug[psb][:, h * 65:h * 65 + 65],
                        start=False, stop=(psb == SB - 1),
                        skip_group_check=True)

            for sb in range(SB):
                ps = psS.tile([128, T], F32, tag="psS", name="psS")
                for tch in range(2):
                    nc.tensor.matmul(
                        ps[:, tch * 512:(tch + 1) * 512],
                        kT[hb][hoff:hoff + 64, sb * 128:(sb + 1) * 128],
                        qT[hb][hoff:hoff + 64, tch * 512:(tch + 1) * 512],
                        start=True, stop=True)
                pO = pb.tile([128, T], BF16, tag="pO", name="pO", bufs=16)
                nc.scalar.activation(pO[:], ps[:], AF.Exp)
                pOs.append(pO)
                sd = sdp.tile([128, T], BF16, tag="sd", name="sd")
                with tc.high_priority():
                    nc.vector.tensor_mul(sd[:], ps[:], dep_t[sb][:])
                sds.append(sd)

                # osa-PV for this s-block into the packed psum tiles
                if sb == 0:
                    for ti, (_, ntb) in enumerate(PV_TILES):
                        ppvs.append(psPV.tile([128, 130 * ntb], F32,
                                              tag=f"ppv{ti}", name=f"ppv{ti}"))
                for tb in range(SB):
                    ti, base = PV_OF[tb]
                    # start=True clears has_written for the WHOLE bank: only
                    # the very first matmul into each psum tile may set it.
                    nc.tensor.matmul(
                        ppvs[ti][:, base:base + 65],
                        pO[:, tb * 128:(tb + 1) * 128],
                        vaug[sb][:, h * 65:h * 65 + 65],
                        start=(sb == 0 and base == 0),
                        stop=(sb == SB - 1),
                        skip_group_check=True)
                # skewed dep branch: exp + dsa-PV for the previous block
                if sb >= 1:
                    emit_dsa_pv(sb - 1)
            emit_dsa_pv(SB - 1)

            # normalize + evict into od[tb] = [osa | dsa] (bf16) on GPSIMD
            for ti, (_, ntb) in enumerate(PV_TILES):
                ppv = ppvs[ti]
                rec = rp.tile([128, 8], F32, tag="rec", name="rec")
                den3 = ppv[:].rearrange("p (g d) -> p g d", d=65)[:, :, 64:65]
                nc.vector.reciprocal(
                    rec[:, 0:2 * ntb].rearrange("p (g d) -> p g d", d=1), den3)
                tb0 = sum(n for _, n in PV_TILES[:ti])
                for k in range(ntb):
                    tb = tb0 + k
                    base = 130 * k
                    out3 = od[tb][:].rearrange(
                        "p (b c) -> p b c", b=2)[:, :, h * 64:(h + 1) * 64]
                    in03 = ppv[:, base:base + 130].rearrange(
                        "p (b c) -> p b c", b=2)[:, :, 0:64]
                    recs = rec[:, 2 * k:2 * k + 2]
                    rec3 = bass.AP(tensor=recs.tensor, offset=recs.offset,
                                   ap=[*recs.ap, [0, 64]])
                    nc.vector.tensor_mul(out3, in03, rec3)

        # ---- stage C: gate + blend in two head-groups. Group 0 (heads 0-5
        # columns) runs during heads 6-11, filling ACT/DVE slack; only group
        # 1 plus the short gate chain sits in the kernel tail. outt runs on
        # GPSIMD (SBUF-only there: no PSUM port on that engine).
        GB = (0, C // 2, C)  # group column bounds per branch: g0 heads 0-5
        z_parts = [[None, None] for _ in range(SB)]
        diffs = [None] * SB

        def stageC_grp(tb, g):
            c0, c1 = GB[g], GB[g + 1]
            w = c1 - c0
            if g == 0:
                diffs[tb] = cp.tile([128, C], BF16, tag="diff", name="diff",
                                    bufs=SB)
            od3 = od[tb][:].rearrange("p (b c) -> p b c", b=2)[:, :, c0:c1]
            tod = cp.tile([128, 2 * w], BF16, tag=f"tod{g}", name="tod")
            tod3 = tod[:].rearrange("p (b c) -> p b c", b=2)
            nc.scalar.activation(tod3, od3, AF.Tanh)
            scr = cp.tile([128, 2 * w], BF16, tag=f"scr{g}", name="scr", bufs=1)
            z = zp.tile([128, 1], F32, tag=f"z{g}", name="z", bufs=SB + 1)
            wg3 = wgb[:].rearrange("p (b c) -> p b c", b=2)[:, :, c0:c1]
            nc.vector.scalar_tensor_tensor(
                out=scr[:].rearrange("p (b c) -> p b c", b=2), in0=tod3,
                scalar=0.0, in1=wg3, op0=ALU.bypass, op1=ALU.mult,
                accum_out=z[:])
            z_parts[tb][g] = z
            nc.gpsimd.tensor_sub(diffs[tb][:, c0:c1], od3[:, 0, :], od3[:, 1, :])

        def stageC_back(tb):
            zt = zp.tile([128, 1], F32, tag="zt", name="zt")
            nc.vector.tensor_add(zt[:], z_parts[tb][0][:], z_parts[tb][1][:])
            e = zp.tile([128, 1], F32, tag="e", name="e")
            # e = exp(-(z + bg)) = exp(-z + nbg)
            nc.scalar.activation(e[:], zt[:], AF.Exp, bias=nbg_t[:], scale=-1.0)
            g = zp.tile([128, 1], F32, tag="g", name="g")
            nc.vector.tensor_scalar_add(e[:], e[:], 1.0)
            nc.vector.reciprocal(g[:], e[:])
            outt = cp.tile([128, C], BF16, tag="outt", name="outt", bufs=4)
            if tb % 2 == 1:
                # offload half the blends: g*diff on ACT (idle after its
                # tanh chain), + dsa on DVE as a 2x-mode bf16 tensor add
                t1 = cp.tile([128, C], BF16, tag="t1", name="t1", bufs=3)
                nc.scalar.activation(t1[:], diffs[tb][:], AF.Copy,
                                     bias=0.0, scale=g[:])
                nc.vector.tensor_add(outt[:], t1[:], od[tb][:, C:2 * C])
            else:
                nc.vector.scalar_tensor_tensor(
                    out=outt[:], in0=diffs[tb][:], scalar=g[:],
                    in1=od[tb][:, C:2 * C], op0=ALU.mult, op1=ALU.add)
            nc.sync.dma_start(out_d[tb * 128:(tb + 1) * 128, :], outt[:])

        # ---- emission: qk(0) through the scores ring (evictions interleaved
        # Qh0, Kh0, Qh1, Kh1 so the first scores matmul unblocks earliest),
        # then all heads with a low-priority feeder batch before each
        ps_qk = [psS.tile([128, T], F32, tag="psS", name=f"psQK{i}")
                 for i in range(2)]
        for kb in range(CB):
            for tch in range(2):
                for i, w in enumerate((wqc, wkc)):
                    nc.tensor.matmul(
                        ps_qk[i][:, tch * 512:(tch + 1) * 512],
                        w[0][:, kb * 128:(kb + 1) * 128],
                        xT[kb][:, tch * 512:(tch + 1) * 512],
                        start=(kb == 0), stop=(kb == CB - 1))
        for tch in range(2):
            # Q half on DVE in parallel with K half on the (idle) ACT engine
            nc.vector.tensor_scalar(
                qT[0][:, tch * 512:(tch + 1) * 512],
                ps_qk[0][:, tch * 512:(tch + 1) * 512], 1.0,
                bqk[:, 0:1], ALU.mult, ALU.add)
            nc.scalar.activation(
                kT[0][:, tch * 512:(tch + 1) * 512],
                ps_qk[1][:, tch * 512:(tch + 1) * 512],
                AF.Identity, bias=bqk[:, CB:CB + 1], scale=1.0)
        for h in range(H):
            if h < len(batches):
                with low_priority():
                    for f in batches[h]:
                        f()
            head(h)
            if h == 5:
                with low_priority():
                    for tb in range(SB):
                        stageC_grp(tb, 0)

        for tb in range(SB):
            stageC_grp(tb, 1)
            if tb >= 1:
                stageC_back(tb - 1)
        stageC_back(SB - 1)

    nc.finalize()
    return nc


_CACHE = {}


def _prep_in_maps(hidden_states, dependency_matrix, Wq, bq, Wk, bk, Wv, bv, Wg, bg):
    hs = np.asarray(hidden_states, dtype=np.float32)
    dep = np.asarray(dependency_matrix, dtype=np.float32)
    bf = ml_dtypes.bfloat16
    shared = {
        # 1/sqrt(D) score scale folded into Wq (exact: power of two)
        "wq": np.ascontiguousarray((np.asarray(Wq, np.float32) * 0.125).T).astype(bf),
        "wk": np.ascontiguousarray(np.asarray(Wk, np.float32).T).astype(bf),
        "wv": np.ascontiguousarray(np.asarray(Wv, np.float32).T).astype(bf),
        "bqs": (np.asarray(bq, np.float32) * 0.125).reshape(C),
        "bk": np.asarray(bk, np.float32).reshape(C),
        "bv": np.ascontiguousarray(np.asarray(bv, np.float32).reshape(C)),
        "wg": np.ascontiguousarray(np.asarray(Wg, np.float32).reshape(2 * C)).astype(bf),
        "nbg": (-np.asarray(bg, np.float32)).reshape(1),
    }
    in_maps = []
    for b in range(B):
        m = dict(shared)
        m["xT"] = np.ascontiguousarray(hs[b].T).astype(bf)
        m["dep"] = np.ascontiguousarray(dep[b].T).astype(bf)
        in_maps.append(m)
    return in_maps


def kernel(**inputs):
    from concourse.bass_utils import run_bass_kernel_spmd
    if "nc" not in _CACHE:
        _CACHE["nc"] = build_nc()
    nc = _CACHE["nc"]
    in_maps = _prep_in_maps(**inputs)
    res = run_bass_kernel_spmd(nc, in_maps, core_ids=list(range(NCORES)))
    out = np.stack([res.results[i]["out"] for i in range(NCORES)], axis=0)
    return out.astype(np.float32)



# revision 54
# speedup vs baseline: 1.1936x; 1.1936x over previous
"""DependencyBertSelfAttention Trainium2 kernel (v5).

Sharding: batch B=8 -> one batch element per NeuronCore (8 cores, SPMD).
Per core (full T=1024, C=768, H=12 heads, D=64):

  Stage B runs as ONE flat software-pipelined loop over all 96 (head,
  s-block) slots with fixed skews: scores+exp at slot i, branch-2 exp at
  i+1, PV-b1 at i+5, PV-b2 at i+4, psum eviction at i+12.  Each
  head-boundary chain (last scores -> pow -> PV stop -> recip/evict ->
  next head's start=True PVs) therefore spreads over ~5 slots and no
  in-order engine head-of-line-blocks on it.

  Branch 1: pO = exp(S^T) on ACT (the only exp-capable engine).
  Branch 2: pd = pow(pO, dep^T) — a single GPSIMD tensor_tensor op
  (software pow, ~0.4% max err) for most slots; a quarter of the slots
  instead run sd = S*dep (DVE) + Schraudolph int16 affine
  round(sd*2^7*log2e + (127*2^7-5.5)) whose int16 bits ARE the bf16
  pattern of exp(sd) (DVE 4x mode), balancing Pool vs DVE load.

  PSUM (8 banks): scores ring psS [128,1024]x2 (4), feeder scratch psA
  [128,512] (1), PV accumulators 3 ([osa|dsa] 65-col pairs per t-block,
  packed 390/390/260; 65th vaug column of ones = softmax denominator).
  PV-weight matmuls must keep out <= 512 f32 (one bank) per instruction.

  Projections: qk(0) through the scores ring up front; remaining Q/K/V
  512-col chunks are low-priority feeders through psA spread over slots
  0-76 (deadline: qk(cb) by slot 16cb, v half-1 by slot 64). V bias is
  added via a k=1 ones-row matmul so the eviction is a pure copy.

  Normalization: one DVE mul per psum tile [p, tb, branch, 64] using
  per-(tb,branch) reciprocals; od packed as one [p, tb, branch, C] tile.

  Stage C: group 0 (heads 0-8 columns) runs during heads 9-11; group 1
  in the tail.  tanh (ACT), z-accum stst (DVE), diff (DVE 2x),
  g = Sigmoid(z+bg) (ACT), blend stst (DVE 4x), DMA out.

No inter-core communication: each core's batch element is independent.
"""
import sys

sys.path.insert(0, "/opt/trn_rl_repo")

import numpy as np
import ml_dtypes
from contextlib import ExitStack

import concourse.bass as bass
import concourse.tile as tile
from concourse import bacc, mybir

B, T, C, H, D = 8, 1024, 768, 12, 64
CB = C // 128   # 6 channel partition-blocks
SB = T // 128   # 8 s/t blocks
NCORES = 8

F32 = mybir.dt.float32
BF16 = mybir.dt.bfloat16
I16 = mybir.dt.int16
AF = mybir.ActivationFunctionType
ALU = mybir.AluOpType

# Schraudolph exp in bf16 bit-space: round(x*2^7*log2e + (127*2^7 - 5.5))
# as int16 == bf16 bit pattern of ~exp(x); the one-sided mantissa-sawtooth
# error cancels in the softmax ratio (measured 8.7e-3 end-to-end rel err).
EXPA = float((1 << 7) * 1.4426950408889634)
EXPB = float(127 * (1 << 7) - 5.5)

# engine assignment knobs (tuned against TimelineSim)
def _b2_eng(h, sb):
    # "pool": pd = pow(pO, dep) — one GPSIMD op, no dep-mul at all.
    # "dve": sd = ps*dep (DVE) + Schraudolph int16 affine (DVE 4x) — used
    # for a fraction of slots to balance Pool vs DVE.
    return "dve" if (h * SB + sb) % 4 == 3 else "pool"


B2_ENG = {(h, sb): _b2_eng(h, sb) for h in range(H) for sb in range(SB)}
Z_G0_POOL = lambda tb: False
QK_ACT = False
SD_BUFS = 4

# PV psum packing: tile index and column base for each t-block
PV_TILES = ((0, 3), (1, 3), (2, 2))          # (tile idx, n t-blocks)
PV_OF = {}
for _ti, (_i, _n) in enumerate(PV_TILES):
    for _k in range(_n):
        PV_OF[sum(n for _, n in PV_TILES[:_ti]) + _k] = (_ti, _k * 130)


def build_nc(debug=False, repeat=1):
    nc = bacc.Bacc("TRN2", target_bir_lowering=False, debug=False,
                   num_devices=NCORES)

    xT_d = nc.dram_tensor("xT", [C, T], BF16, kind="ExternalInput").ap()
    wq_d = nc.dram_tensor("wq", [C, C], BF16, kind="ExternalInput").ap()
    wk_d = nc.dram_tensor("wk", [C, C], BF16, kind="ExternalInput").ap()
    wv_d = nc.dram_tensor("wv", [C, C], BF16, kind="ExternalInput").ap()
    bqs_d = nc.dram_tensor("bqs", [C], F32, kind="ExternalInput").ap()
    bk_d = nc.dram_tensor("bk", [C], F32, kind="ExternalInput").ap()
    bvh_d = nc.dram_tensor("bvh", [C], BF16, kind="ExternalInput").ap()
    dep_d = nc.dram_tensor("dep", [T, T], BF16, kind="ExternalInput").ap()
    wg_d = nc.dram_tensor("wg", [2 * C], BF16, kind="ExternalInput").ap()
    nbg_d = nc.dram_tensor("nbg", [1], F32, kind="ExternalInput").ap()
    out_d = nc.dram_tensor("out", [T, C], BF16, kind="ExternalOutput").ap()
    dbg_d = (nc.dram_tensor("dbg", [128, SB * 2 * C], BF16,
                            kind="ExternalOutput").ap() if debug else None)
    dbg2_d = (nc.dram_tensor("dbg2", [128, T], BF16,
                             kind="ExternalOutput").ap() if debug else None)

    def bcast(src_ap, n_free):
        return bass.AP(tensor=src_ap.tensor, offset=src_ap.offset,
                       ap=[[0, 128], [1, n_free]])

    def chunked(src_ap, cb0, ncb, width):
        # dram [C, width] rows cb0*128.. viewed as [p, cb, w] for one DMA
        return bass.AP(tensor=src_ap.tensor, offset=src_ap.offset + cb0 * 128 * width,
                       ap=[[width, 128], [128 * width, ncb], [1, width]])

    def col_chunk(src_ap, cb):
        # dram [C, C] columns cb*128..: [p, kb, j] = src[kb*128+p, cb*128+j]
        return bass.AP(tensor=src_ap.tensor, offset=src_ap.offset + cb * 128,
                       ap=[[C, 128], [128 * C, CB], [1, 128]])

    def col_fold(src_ap, ncb):
        # dram [C] -> [128, ncb]: partition p, col cb = src[cb*128 + p]
        return bass.AP(tensor=src_ap.tensor, offset=src_ap.offset,
                       ap=[[1, 128], [128, ncb]])

    with tile.TileContext(nc, pool_alloc_mode="queue") as tc, ExitStack() as ctx:
      persist = ctx.enter_context(tc.tile_pool(name="persist", bufs=1))
      psS = ctx.enter_context(tc.tile_pool(name="psS", bufs=2, space="PSUM"))
      psA = ctx.enter_context(tc.tile_pool(name="psA", bufs=1, space="PSUM"))
      psPV = ctx.enter_context(tc.tile_pool(name="psPV", bufs=1, space="PSUM"))
      for _rep in range(repeat):
        # ---- persistent tiles
        xTa = persist.tile([128, CB * T], BF16, tag="xTa", name="xTa")
        # weights as per-output-cb column tiles: wqc[cb][p, kb*128+j] =
        # Wq.T[kb*128+p, cb*128+j] — head 0 needs only cb=0 (196KB, not 1.2MB)
        wqc = [persist.tile([128, CB * 128], BF16, tag=f"wqc{i}", name=f"wqc{i}")
               for i in range(CB)]
        wkc = [persist.tile([128, CB * 128], BF16, tag=f"wkc{i}", name=f"wkc{i}")
               for i in range(CB)]
        wva = persist.tile([128, CB * C], BF16, tag="wva", name="wva")
        xT = [xTa[:, i * T:(i + 1) * T] for i in range(CB)]
        wvs = [wva[:, i * C:(i + 1) * C] for i in range(CB)]
        qT = [persist.tile([128, T], BF16, tag=f"qT{i}", name=f"qT{i}") for i in range(CB)]
        kT = [persist.tile([128, T], BF16, tag=f"kT{i}", name=f"kT{i}") for i in range(CB)]
        vaug = [persist.tile([128, H * 65], BF16, tag=f"vaug{i}", name=f"vaug{i}") for i in range(SB)]
        dep_t = [persist.tile([128, T], BF16, tag=f"dep{i}", name=f"dep{i}") for i in range(SB)]
        # od packed as one tile [p, tb, branch, C] so per-head normalization
        # evictions can cover all t-blocks of a psum tile in one instruction
        od_all = persist.tile([128, SB * 2 * C], BF16, tag="od", name="od")
        od = [od_all[:, i * 2 * C:(i + 1) * 2 * C] for i in range(SB)]
        bqk = persist.tile([128, 2 * CB], F32, tag="bqk", name="bqk")
        ones1 = persist.tile([1, T], BF16, tag="ones1", name="ones1")
        bvr = persist.tile([1, C], BF16, tag="bvr", name="bvr")
        wgb = persist.tile([128, 2 * C], BF16, tag="wgb", name="wgb")
        nbg_t = persist.tile([128, 1], F32, tag="nbg", name="nbg")

        # warm the ACT exp/tanh table set at t=0 so the first real exp
        # doesn't pay the ~2.7us table load mid-ramp; keep PE continuously
        # busy with dummy matmuls so its p-state ramps to full clock before
        # the first projection matmuls arrive (~6us in)
        with tc.high_priority():
            warm = persist.tile([128, 1], F32, tag="warm", name="warm")
            nc.vector.memset(warm[:], 0.0)
            nc.scalar.activation(warm[:], warm[:], AF.Exp)
            zz = persist.tile([128, 512], BF16, tag="zz", name="zz")
            nc.vector.memset(zz[:], 0.0)
            wz = persist.tile([128, 1], BF16, tag="wz", name="wz")
            nc.vector.memset(wz[:], 0.0)
            for _ in range(8):
                pwarm = psS.tile([128, T], F32, tag="psS", name="pwarm")
                nc.tensor.matmul(pwarm[0:1, 0:512], wz[:], zz[:],
                                 start=True, stop=True)

        # ---- DMAs on the sync queue: the serialized DMA engines process
        # transfers in exactly this order, so startup-critical data (x, the
        # cb=0 weight columns) comes first.
        hc = CB // 2
        def wcol(dst, src, cb):
            nc.sync.dma_start(
                dst[cb][:].rearrange("p (kb j) -> p kb j", j=128),
                col_chunk(src, cb))
        nc.sync.dma_start(
            xTa[:, 0:2 * T].rearrange("p (cb t) -> p cb t", t=T),
            chunked(xT_d, 0, 2, T))
        wcol(wqc, wq_d, 0)
        wcol(wkc, wk_d, 0)
        nc.sync.dma_start(
            xTa[:, 2 * T:4 * T].rearrange("p (cb t) -> p cb t", t=T),
            chunked(xT_d, 2, 2, T))
        nc.sync.dma_start(
            xTa[:, 4 * T:CB * T].rearrange("p (cb t) -> p cb t", t=T),
            chunked(xT_d, 4, 2, T))
        nc.sync.dma_start(bqk[:, 0:CB], col_fold(bqs_d, CB))
        nc.sync.dma_start(bqk[:, CB:2 * CB], col_fold(bk_d, CB))
        # wva before dep: the v feeders gate head 0's PVs, while dep is now
        # consumed one slot later (by the pow) than the scores
        nc.sync.dma_start(wva[:].rearrange("p (cb c) -> p cb c", c=C),
                          chunked(wv_d, 0, CB, C))
        nc.sync.dma_start(bvr[:], bass.AP(tensor=bvh_d.tensor, offset=bvh_d.offset,
                                          ap=[[0, 1], [1, C]]))
        nc.vector.memset(ones1[:], 1.0)
        nc.sync.dma_start(dep_t[0][:], dep_d[0:128, :])
        nc.sync.dma_start(dep_t[1][:], dep_d[128:256, :])
        for cb in range(1, CB):
            wcol(wqc, wq_d, cb)
            wcol(wkc, wk_d, cb)
            nc.sync.dma_start(dep_t[cb + 1][:],
                              dep_d[(cb + 1) * 128:(cb + 2) * 128, :])
        nc.sync.dma_start(dep_t[7][:], dep_d[7 * 128:8 * 128, :])
        nc.sync.dma_start(wgb[:], bcast(wg_d, 2 * C))
        nc.sync.dma_start(nbg_t[:], bcast(nbg_d, 1))

        # ---- stage-A emission units
        def proj_one(dst, w, cb, bcol, eng):
            # full Q^T or K^T chunk through the scores ring (used for qk(0)
            # ahead of head 0 only): kb-outer so matmuls start as DMA chunks
            # land; per-half evictions unblock the first scores early.
            ps = psS.tile([128, T], F32, tag="psS", name="psS")
            for kb in range(CB):
                for tch in range(2):
                    nc.tensor.matmul(
                        ps[:, tch * 512:(tch + 1) * 512],
                        w[cb][:, kb * 128:(kb + 1) * 128],
                        xT[kb][:, tch * 512:(tch + 1) * 512],
                        start=(kb == 0), stop=(kb == CB - 1))
            for tch in range(2):
                eng.tensor_scalar(
                    dst[cb][:, tch * 512:(tch + 1) * 512],
                    ps[:, tch * 512:(tch + 1) * 512], 1.0,
                    bqk[:, bcol:bcol + 1], ALU.mult, ALU.add)

        def qk_half(dst, w, cb, bcol, tch):
            # 512-col half of a Q^T/K^T chunk through the 1-bank psA scratch
            ps = psA.tile([128, 512], F32, tag="psA", name="psA")
            for kb in range(CB):
                nc.tensor.matmul(
                    ps[:], w[cb][:, kb * 128:(kb + 1) * 128],
                    xT[kb][:, tch * 512:(tch + 1) * 512],
                    start=(kb == 0), stop=(kb == CB - 1))
            with tc.high_priority():
                if QK_ACT:
                    nc.scalar.activation(
                        dst[cb][:, tch * 512:(tch + 1) * 512], ps[:],
                        AF.Identity, bias=bqk[:, bcol:bcol + 1], scale=1.0)
                else:
                    nc.vector.tensor_scalar(
                        dst[cb][:, tch * 512:(tch + 1) * 512], ps[:], 1.0,
                        bqk[:, bcol:bcol + 1], ALU.mult, ALU.add)

        def v_half(sb, half):
            # V natural [s part, c free] 512/256-col chunk -> v_aug.  The
            # bias lands via a k=1 ones-row matmul so the eviction is a
            # pure copy (DVE).
            n0, nw = (0, 512) if half == 0 else (512, 256)
            ps = psA.tile([128, 512], F32, tag="psA", name="psA_v")
            for kb in range(CB):
                nc.tensor.matmul(
                    ps[:, 0:nw],
                    xT[kb][:, sb * 128:(sb + 1) * 128],
                    wvs[kb][:, n0:n0 + nw],
                    start=(kb == 0), stop=False)
            nc.tensor.matmul(
                ps[:, 0:nw], ones1[:, sb * 128:(sb + 1) * 128],
                bvr[:, n0:n0 + nw], start=False, stop=True)
            va3 = vaug[sb][:].rearrange("p (h d) -> p h d", d=65)
            h0 = n0 // 64
            ps3 = ps[:, 0:nw].rearrange("p (h d) -> p h d", d=64)
            with tc.high_priority():
                nc.scalar.activation(va3[:, h0:h0 + nw // 64, 0:64], ps3,
                                     AF.Copy)
            if half == 0:
                nc.vector.memset(va3[:, :, 64:65], 1.0)

        # Stage-A feeders are emitted BEFORE their consumers (emission order
        # is semantic order in Tile: a reader emitted before its writer reads
        # garbage), but with the priority counter pushed past every normal
        # instruction, so the scheduler only runs them in engine idle slots.
        from contextlib import contextmanager
        feeder_prio = [1 << 20]

        @contextmanager
        def low_priority():
            orig = tc.cur_priority
            tc.cur_priority = feeder_prio[0]
            try:
                yield
            finally:
                feeder_prio[0] = tc.cur_priority
                tc.cur_priority = orig

        def qk_feed(cb, qk, t):
            return lambda: qk_half((qT, kT)[qk], (wqc, wkc)[qk],
                                   cb, qk * CB + cb, t)
        def v_feed(sb, hf):
            return lambda: v_half(sb, hf)
        # feed_slots[i] = feeder thunks emitted at flat-loop slot i, spread
        # so the PE feeder load is ~flat until head 9.  Deadlines: vaug[sb]
        # half-0 by slot sb+5, qk(cb) by slot 16*cb, vaug half-1 by slot 69.
        feed_slots = {}

        def add_feed(slot, fn):
            feed_slots.setdefault(slot, []).append(fn)

        for sb in range(SB):
            add_feed(sb, v_feed(sb, 0))
        for k, (qk, t) in enumerate((q, t) for q in range(2) for t in range(2)):
            add_feed(8 + 2 * k, qk_feed(1, qk, t))
            add_feed(16 + 4 * k, qk_feed(2, qk, t))
            add_feed(32 + 4 * k, qk_feed(3, qk, t))
            add_feed(47 + 4 * k, qk_feed(4, qk, t))
            add_feed(69 + 2 * k, qk_feed(5, qk, t))
        for sb in range(SB):
            add_feed(60 + sb, v_feed(sb, 1))

        pb = ctx.enter_context(tc.tile_pool(name="pP", bufs=7))
        pdp = ctx.enter_context(tc.tile_pool(name="pdp", bufs=5))
        sdp = ctx.enter_context(tc.tile_pool(name="sdp", bufs=3))
        rp = ctx.enter_context(tc.tile_pool(name="rp", bufs=16))
        cp = ctx.enter_context(tc.tile_pool(name="stageC", bufs=2))
        zp = ctx.enter_context(tc.tile_pool(name="zP", bufs=3))

        # ---- stage B: flat software-pipelined loop over all (h, sb) slots.
        # Skews (in slots): affine +1, PV-b1 +5, PV-b2 +4, psum evict +12.
        # This spreads each head-boundary chain (mul7 -> affine7 -> PV-b2(7)
        # -> recips/evicts -> next head's start=True PVs) across ~5 slots so
        # no in-order engine ever head-of-line-blocks on it.
        NS = H * SB
        pO_l = [None] * NS
        sd_l = [None] * NS
        pD_l = [None] * NS
        ppvh = [None] * H

        def slot_scores(i):
            h, sb = divmod(i, SB)
            hb, hoff = h // 2, (h % 2) * 64
            ps = psS.tile([128, T], F32, tag="psS", name="psS")
            for tch in range(2):
                nc.tensor.matmul(
                    ps[:, tch * 512:(tch + 1) * 512],
                    kT[hb][hoff:hoff + 64, sb * 128:(sb + 1) * 128],
                    qT[hb][hoff:hoff + 64, tch * 512:(tch + 1) * 512],
                    start=True, stop=True)
            pO = pb.tile([128, T], BF16, tag="pO", name="pO", bufs=12)
            nc.scalar.activation(pO[:], ps[:], AF.Exp)
            pO_l[i] = pO
            if B2_ENG.get((h, sb)) == "dve":
                sd = sdp.tile([128, T], BF16, tag="sd", name="sd",
                              bufs=SD_BUFS)
                with tc.high_priority():
                    nc.vector.tensor_mul(sd[:], ps[:], dep_t[sb][:])
                sd_l[i] = sd

        def slot_affine(i):
            # dep-branch exp: Pool slots compute pd = pow(pO, dep) directly
            # (GPSIMD software pow, ~0.4% max err); DVE slots use the
            # Schraudolph int16 affine on sd = s*dep (4x mode).
            h, sb = divmod(i, SB)
            eng = B2_ENG.get((h, sb), "pool")
            if eng == "pool":
                pD = pdp.tile([128, T], BF16, tag="pD", name="pD", bufs=6)
                nc.gpsimd.tensor_tensor(pD[:], pO_l[i][:], dep_t[sb][:],
                                        ALU.pow)
                pD_l[i] = pD[:]
                if dbg2_d is not None and i == 0:
                    nc.sync.dma_start(dbg2_d, pD[:])
            else:
                pD = pdp.tile([128, T], I16, tag="pDi", name="pDi", bufs=4)
                nc.vector.tensor_scalar(pD[:], sd_l[i][:], EXPA, EXPB,
                                        ALU.mult, ALU.add)
                pD_l[i] = pD[:].bitcast(BF16)

        def slot_pv(i, br):
            h, sb = divmod(i, SB)
            if ppvh[h] is None:
                ppvh[h] = [psPV.tile([128, 130 * ntb], F32,
                                     tag=f"ppv{ti}", name=f"ppv{ti}")
                           for ti, (_, ntb) in enumerate(PV_TILES)]
            px = pO_l[i][:] if br == 0 else pD_l[i]
            for tb in range(SB):
                ti, base = PV_OF[tb]
                lhsT = px[:, tb * 128:(tb + 1) * 128]
                nc.tensor.matmul(
                    ppvh[h][ti][:, base + 65 * br:base + 65 * br + 65],
                    lhsT, vaug[sb][:, h * 65:h * 65 + 65],
                    # start=True clears has_written for the WHOLE bank: only
                    # the very first matmul into each psum tile may set it —
                    # that is the b2(sb0) group (emitted at slot h*8+4, one
                    # before b1(sb0)).
                    start=(br == 1 and sb == 0 and base == 0),
                    stop=(sb == SB - 1),
                    skip_group_check=True)

        def evict_head(h):
            # normalize + evict into od[tb] = [osa | dsa] (bf16): one DVE
            # mul per psum tile covering all its t-blocks [p, tb, br, 64]
            for ti, (_, ntb) in enumerate(PV_TILES):
                ppv = ppvh[h][ti]
                rec = rp.tile([128, 8], F32, tag="rec", name="rec")
                den3 = ppv[:].rearrange("p (g d) -> p g d", d=65)[:, :, 64:65]
                nc.vector.reciprocal(
                    rec[:, 0:2 * ntb].rearrange("p (g d) -> p g d", d=1), den3)
                tb0 = sum(n for _, n in PV_TILES[:ti])
                out4 = od_all[:, tb0 * 2 * C:(tb0 + ntb) * 2 * C].rearrange(
                    "p (t b c) -> p t b c", b=2, c=C)[:, :, :, h * 64:(h + 1) * 64]
                in4 = ppv[:].rearrange(
                    "p (t b d) -> p t b d", b=2, d=65)[:, :, :, 0:64]
                recs = rec[:, 0:2 * ntb]
                rec4 = bass.AP(
                    tensor=recs.tensor, offset=recs.offset,
                    ap=[recs.ap[0], [2, ntb], [1, 2], [0, 64]])
                nc.vector.tensor_mul(out4, in4, rec4)

        # ---- stage C: gate + blend in two head-groups. Group 0 (heads 0-8
        # columns) runs during heads 9-11, filling engine slack; only group
        # 1 (3 heads) plus the short gate chain sits in the kernel tail.
        # z-accumulation: g0 on Pool, g1 on DVE (idle in the tail).
        GB = (0, 3 * C // 4, C)  # group column bounds per branch: g0 heads 0-8
        z_parts = [[None, None] for _ in range(SB)]
        diffs = [None] * SB

        def stageC_grp(tb, g):
            c0, c1 = GB[g], GB[g + 1]
            w = c1 - c0
            if g == 0:
                diffs[tb] = cp.tile([128, C], BF16, tag="diff", name="diff",
                                    bufs=SB)
            od3 = od[tb].rearrange("p (b c) -> p b c", b=2)[:, :, c0:c1]
            tod = cp.tile([128, 2 * w], BF16, tag=f"tod{g}", name="tod")
            tod3 = tod[:].rearrange("p (b c) -> p b c", b=2)
            nc.scalar.activation(tod3, od3, AF.Tanh)
            scr = cp.tile([128, 2 * w], BF16, tag=f"scr{g}", name="scr", bufs=1)
            z = zp.tile([128, 1], F32, tag=f"z{g}", name="z", bufs=SB + 1)
            wg3 = wgb[:].rearrange("p (b c) -> p b c", b=2)[:, :, c0:c1]
            zeng = nc.gpsimd if (g == 0 and Z_G0_POOL(tb)) else nc.vector
            zeng.scalar_tensor_tensor(
                out=scr[:].rearrange("p (b c) -> p b c", b=2), in0=tod3,
                scalar=0.0, in1=wg3, op0=ALU.bypass, op1=ALU.mult,
                accum_out=z[:])
            z_parts[tb][g] = z
            nc.vector.tensor_sub(diffs[tb][:, c0:c1], od3[:, 0, :], od3[:, 1, :])

        def stageC_back(tb):
            zt = zp.tile([128, 1], F32, tag="zt", name="zt")
            nc.vector.tensor_add(zt[:], z_parts[tb][0][:], z_parts[tb][1][:])
            g = zp.tile([128, 1], F32, tag="g", name="g")
            # g = sigmoid(z + bg) in one ACT op (bias = +bg, so negate nbg)
            nc.scalar.activation(g[:], zt[:], AF.Sigmoid, bias=nbg_t[:],
                                 scale=1.0)
            outt = cp.tile([128, C], BF16, tag="outt", name="outt", bufs=4)
            # all-bf16 SBUF stst (no accum) hits DVE 4x mode: ~260ns
            nc.vector.scalar_tensor_tensor(
                out=outt[:], in0=diffs[tb][:], scalar=g[:],
                in1=od[tb][:, C:2 * C], op0=ALU.mult, op1=ALU.add)
            nc.sync.dma_start(out_d[tb * 128:(tb + 1) * 128, :], outt[:])

        # ---- emission: qk(0) through the scores ring (evictions interleaved
        # Qh0, Kh0, Qh1, Kh1 so the first scores matmul unblocks earliest),
        # then all heads with a low-priority feeder batch before each
        ps_qk = [psS.tile([128, T], F32, tag="psS", name=f"psQK{i}")
                 for i in range(2)]
        for kb in range(CB):
            for tch in range(2):
                for i, w in enumerate((wqc, wkc)):
                    nc.tensor.matmul(
                        ps_qk[i][:, tch * 512:(tch + 1) * 512],
                        w[0][:, kb * 128:(kb + 1) * 128],
                        xT[kb][:, tch * 512:(tch + 1) * 512],
                        start=(kb == 0), stop=(kb == CB - 1))
        for tch in range(2):
            # Q half on DVE in parallel with K half on the (idle) ACT engine
            nc.vector.tensor_scalar(
                qT[0][:, tch * 512:(tch + 1) * 512],
                ps_qk[0][:, tch * 512:(tch + 1) * 512], 1.0,
                bqk[:, 0:1], ALU.mult, ALU.add)
            nc.scalar.activation(
                kT[0][:, tch * 512:(tch + 1) * 512],
                ps_qk[1][:, tch * 512:(tch + 1) * 512],
                AF.Identity, bias=bqk[:, CB:CB + 1], scale=1.0)
        for i in range(NS + 13):
            if i < NS:
                slot_scores(i)
            if i in feed_slots:
                with low_priority():
                    for f in feed_slots[i]:
                        f()
            if i >= 77 and (i - 77) % 2 == 0 and (i - 77) // 2 < SB:
                # heads 0-8 od columns complete (evict(8) at slot 76);
                # spread g0 so its z work doesn't jam the Pool queue
                with low_priority():
                    stageC_grp((i - 77) // 2, 0)
            if 1 <= i <= NS and i - 1 < NS:
                slot_affine(i - 1)
            # slot h*8+4 carries, in order: b1(h-1, sb7) (completes head
            # h-1's accumulation), evict(h-1) (reads the full bank), then
            # b2(h, sb0) with start=True (clobbers the bank for head h)
            if i >= 5 and i - 5 < NS:
                slot_pv(i - 5, 0)
            if i >= 12 and (i - 12) % SB == 0 and (i - 12) // SB < H:
                evict_head((i - 12) // SB)
            if i >= 4 and i - 4 < NS:
                slot_pv(i - 4, 1)

        for tb in range(SB):
            stageC_grp(tb, 1)
            if tb >= 1:
                stageC_back(tb - 1)
        stageC_back(SB - 1)
        if debug:
            nc.sync.dma_start(dbg_d, od_all[:])

    nc.finalize()
    return nc


_CACHE = {}


def _prep_in_maps(hidden_states, dependency_matrix, Wq, bq, Wk, bk, Wv, bv, Wg, bg):
    hs = np.asarray(hidden_states, dtype=np.float32)
    dep = np.asarray(dependency_matrix, dtype=np.float32)
    bf = ml_dtypes.bfloat16
    shared = {
        # 1/sqrt(D) score scale folded into Wq (exact: power of two)
        "wq": np.ascontiguousarray((np.asarray(Wq, np.float32) * 0.125).T).astype(bf),
        "wk": np.ascontiguousarray(np.asarray(Wk, np.float32).T).astype(bf),
        "wv": np.ascontiguousarray(np.asarray(Wv, np.float32).T).astype(bf),
        "bqs": (np.asarray(bq, np.float32) * 0.125).reshape(C),
        "bk": np.asarray(bk, np.float32).reshape(C),
        "bvh": np.ascontiguousarray(np.asarray(bv, np.float32).reshape(C)).astype(bf),
        "wg": np.ascontiguousarray(np.asarray(Wg, np.float32).reshape(2 * C)).astype(bf),
        # "nbg" now carries +bg (Sigmoid bias), name kept for dram binding
        "nbg": np.asarray(bg, np.float32).reshape(1),
    }
    in_maps = []
    for b in range(B):
        m = dict(shared)
        m["xT"] = np.ascontiguousarray(hs[b].T).astype(bf)
        m["dep"] = np.ascontiguousarray(dep[b].T).astype(bf)
        in_maps.append(m)
    return in_maps


def kernel(**inputs):
    from concourse.bass_utils import run_bass_kernel_spmd
    if "nc" not in _CACHE:
        _CACHE["nc"] = build_nc()
    nc = _CACHE["nc"]
    in_maps = _prep_in_maps(**inputs)
    res = run_bass_kernel_spmd(nc, in_maps, core_ids=list(range(NCORES)))
    out = np.stack([res.results[i]["out"] for i in range(NCORES)], axis=0)
    return out.astype(np.float32)



# revision 62
# speedup vs baseline: 1.1999x; 1.0053x over previous
"""DependencyBertSelfAttention Trainium2 kernel (v5).

Sharding: batch B=8 -> one batch element per NeuronCore (8 cores, SPMD).
Per core (full T=1024, C=768, H=12 heads, D=64):

  Stage B runs as ONE flat software-pipelined loop over all 96 (head,
  s-block) slots with fixed skews: scores+exp at slot i, branch-2 exp at
  i+1, PV-b1 at i+5, PV-b2 at i+4, psum eviction at i+12.  Each
  head-boundary chain (last scores -> pow -> PV stop -> recip/evict ->
  next head's start=True PVs) therefore spreads over ~5 slots and no
  in-order engine head-of-line-blocks on it.

  Branch 1: pO = exp(S^T) on ACT (the only exp-capable engine).
  Branch 2: pd = pow(pO, dep^T) — a single GPSIMD tensor_tensor op
  (software pow, ~0.4% max err) for most slots; a quarter of the slots
  instead run sd = S*dep (DVE) + Schraudolph int16 affine
  round(sd*2^7*log2e + (127*2^7-5.5)) whose int16 bits ARE the bf16
  pattern of exp(sd) (DVE 4x mode), balancing Pool vs DVE load.

  PSUM (8 banks): scores ring psS [128,1024]x2 (4), feeder scratch psA
  [128,512] (1), PV accumulators 3 ([osa|dsa] 65-col pairs per t-block,
  packed 390/390/260; 65th vaug column of ones = softmax denominator).
  PV-weight matmuls must keep out <= 512 f32 (one bank) per instruction.

  Projections: qk(0) through the scores ring up front; remaining Q/K/V
  512-col chunks are low-priority feeders through psA spread over slots
  0-76 (deadline: qk(cb) by slot 16cb, v half-1 by slot 64). V bias is
  added via a k=1 ones-row matmul so the eviction is a pure copy.

  Normalization: one DVE mul per psum tile [p, tb, branch, 64] using
  per-(tb,branch) reciprocals; od packed as one [p, tb, branch, C] tile.

  Stage C in three column groups so only head 11's columns sit in the
  kernel tail: g0 (heads 0-8) from slot 77, g0b (heads 9-10) from slot
  93, g1 (head 11) after the flat loop.  Per (tb, group): tanh (ACT),
  z-accum stst (DVE), diff (DVE 2x); then zt = z0+z0b+z1,
  g = Sigmoid(zt+bg) (ACT), blend stst, DMA out.

No inter-core communication: each core's batch element is independent.
"""
import sys

sys.path.insert(0, "/opt/trn_rl_repo")

import numpy as np
import ml_dtypes
from contextlib import ExitStack

import concourse.bass as bass
import concourse.tile as tile
from concourse import bacc, mybir

B, T, C, H, D = 8, 1024, 768, 12, 64
CB = C // 128   # 6 channel partition-blocks
SB = T // 128   # 8 s/t blocks
NCORES = 8

F32 = mybir.dt.float32
BF16 = mybir.dt.bfloat16
I16 = mybir.dt.int16
AF = mybir.ActivationFunctionType
ALU = mybir.AluOpType

# Schraudolph exp in bf16 bit-space: round(x*2^7*log2e + (127*2^7 - 5.5))
# as int16 == bf16 bit pattern of ~exp(x); the one-sided mantissa-sawtooth
# error cancels in the softmax ratio (measured 8.7e-3 end-to-end rel err).
EXPA = float((1 << 7) * 1.4426950408889634)
EXPB = float(127 * (1 << 7) - 5.5)

# engine assignment knobs (tuned against TimelineSim)
def _b2_eng(h, sb):
    # "pool": pd = pow(pO, dep) — one GPSIMD op, no dep-mul at all.
    # "dve": sd = ps*dep (DVE) + Schraudolph int16 affine (DVE 4x) — used
    # for a fraction of slots to balance Pool vs DVE.
    return "dve" if (h * SB + sb) % 4 == 3 else "pool"


B2_ENG = {(h, sb): _b2_eng(h, sb) for h in range(H) for sb in range(SB)}
Z_G0_POOL = lambda tb: False
QK_ACT = False
SD_BUFS = 4

# PV psum packing: tile index and column base for each t-block
PV_TILES = ((0, 3), (1, 3), (2, 2))          # (tile idx, n t-blocks)
PV_OF = {}
for _ti, (_i, _n) in enumerate(PV_TILES):
    for _k in range(_n):
        PV_OF[sum(n for _, n in PV_TILES[:_ti]) + _k] = (_ti, _k * 130)


def build_nc(debug=False, repeat=1):
    nc = bacc.Bacc("TRN2", target_bir_lowering=False, debug=False,
                   num_devices=NCORES)

    xT_d = nc.dram_tensor("xT", [C, T], BF16, kind="ExternalInput").ap()
    wq_d = nc.dram_tensor("wq", [C, C], BF16, kind="ExternalInput").ap()
    wk_d = nc.dram_tensor("wk", [C, C], BF16, kind="ExternalInput").ap()
    wv_d = nc.dram_tensor("wv", [C, C], BF16, kind="ExternalInput").ap()
    bqs_d = nc.dram_tensor("bqs", [C], F32, kind="ExternalInput").ap()
    bk_d = nc.dram_tensor("bk", [C], F32, kind="ExternalInput").ap()
    bvh_d = nc.dram_tensor("bvh", [C], BF16, kind="ExternalInput").ap()
    dep_d = nc.dram_tensor("dep", [T, T], BF16, kind="ExternalInput").ap()
    wg_d = nc.dram_tensor("wg", [2 * C], BF16, kind="ExternalInput").ap()
    nbg_d = nc.dram_tensor("nbg", [1], F32, kind="ExternalInput").ap()
    out_d = nc.dram_tensor("out", [T, C], BF16, kind="ExternalOutput").ap()
    dbg_d = (nc.dram_tensor("dbg", [128, SB * 2 * C], BF16,
                            kind="ExternalOutput").ap() if debug else None)
    dbg2_d = (nc.dram_tensor("dbg2", [128, T], BF16,
                             kind="ExternalOutput").ap() if debug else None)

    def bcast(src_ap, n_free):
        return bass.AP(tensor=src_ap.tensor, offset=src_ap.offset,
                       ap=[[0, 128], [1, n_free]])

    def chunked(src_ap, cb0, ncb, width):
        # dram [C, width] rows cb0*128.. viewed as [p, cb, w] for one DMA
        return bass.AP(tensor=src_ap.tensor, offset=src_ap.offset + cb0 * 128 * width,
                       ap=[[width, 128], [128 * width, ncb], [1, width]])

    def col_chunk(src_ap, cb):
        # dram [C, C] columns cb*128..: [p, kb, j] = src[kb*128+p, cb*128+j]
        return bass.AP(tensor=src_ap.tensor, offset=src_ap.offset + cb * 128,
                       ap=[[C, 128], [128 * C, CB], [1, 128]])

    def col_fold(src_ap, ncb):
        # dram [C] -> [128, ncb]: partition p, col cb = src[cb*128 + p]
        return bass.AP(tensor=src_ap.tensor, offset=src_ap.offset,
                       ap=[[1, 128], [128, ncb]])

    with tile.TileContext(nc, pool_alloc_mode="queue") as tc, ExitStack() as ctx:
      persist = ctx.enter_context(tc.tile_pool(name="persist", bufs=1))
      psS = ctx.enter_context(tc.tile_pool(name="psS", bufs=2, space="PSUM"))
      psA = ctx.enter_context(tc.tile_pool(name="psA", bufs=1, space="PSUM"))
      psPV = ctx.enter_context(tc.tile_pool(name="psPV", bufs=1, space="PSUM"))
      for _rep in range(repeat):
        # ---- persistent tiles
        xTa = persist.tile([128, CB * T], BF16, tag="xTa", name="xTa")
        # weights as per-output-cb column tiles: wqc[cb][p, kb*128+j] =
        # Wq.T[kb*128+p, cb*128+j] — head 0 needs only cb=0 (196KB, not 1.2MB)
        wqc = [persist.tile([128, CB * 128], BF16, tag=f"wqc{i}", name=f"wqc{i}")
               for i in range(CB)]
        wkc = [persist.tile([128, CB * 128], BF16, tag=f"wkc{i}", name=f"wkc{i}")
               for i in range(CB)]
        wva = persist.tile([128, CB * C], BF16, tag="wva", name="wva")
        xT = [xTa[:, i * T:(i + 1) * T] for i in range(CB)]
        wvs = [wva[:, i * C:(i + 1) * C] for i in range(CB)]
        qT = [persist.tile([128, T], BF16, tag=f"qT{i}", name=f"qT{i}") for i in range(CB)]
        kT = [persist.tile([128, T], BF16, tag=f"kT{i}", name=f"kT{i}") for i in range(CB)]
        vaug = [persist.tile([128, H * 65], BF16, tag=f"vaug{i}", name=f"vaug{i}") for i in range(SB)]
        dep_t = [persist.tile([128, T], BF16, tag=f"dep{i}", name=f"dep{i}") for i in range(SB)]
        # od packed as one tile [p, tb, branch, C] so per-head normalization
        # evictions can cover all t-blocks of a psum tile in one instruction
        od_all = persist.tile([128, SB * 2 * C], BF16, tag="od", name="od")
        od = [od_all[:, i * 2 * C:(i + 1) * 2 * C] for i in range(SB)]
        bqk = persist.tile([128, 2 * CB], F32, tag="bqk", name="bqk")
        ones1 = persist.tile([1, T], BF16, tag="ones1", name="ones1")
        bvr = persist.tile([1, C], BF16, tag="bvr", name="bvr")
        wgb = persist.tile([128, 2 * C], BF16, tag="wgb", name="wgb")
        nbg_t = persist.tile([128, 1], F32, tag="nbg", name="nbg")

        # warm the ACT exp/tanh table set at t=0 so the first real exp
        # doesn't pay the ~2.7us table load mid-ramp; keep PE continuously
        # busy with dummy matmuls so its p-state ramps to full clock before
        # the first projection matmuls arrive (~6us in)
        with tc.high_priority():
            warm = persist.tile([128, 1], F32, tag="warm", name="warm")
            nc.vector.memset(warm[:], 0.0)
            nc.scalar.activation(warm[:], warm[:], AF.Exp)
            zz = persist.tile([128, 512], BF16, tag="zz", name="zz")
            nc.vector.memset(zz[:], 0.0)
            wz = persist.tile([128, 1], BF16, tag="wz", name="wz")
            nc.vector.memset(wz[:], 0.0)
            for _ in range(8):
                pwarm = psS.tile([128, T], F32, tag="psS", name="pwarm")
                nc.tensor.matmul(pwarm[0:1, 0:512], wz[:], zz[:],
                                 start=True, stop=True)

        # ---- DMAs on the sync queue: the serialized DMA engines process
        # transfers in exactly this order, so startup-critical data (x, the
        # cb=0 weight columns) comes first.
        hc = CB // 2
        def wcol(dst, src, cb):
            nc.sync.dma_start(
                dst[cb][:].rearrange("p (kb j) -> p kb j", j=128),
                col_chunk(src, cb))
        nc.sync.dma_start(
            xTa[:, 0:2 * T].rearrange("p (cb t) -> p cb t", t=T),
            chunked(xT_d, 0, 2, T))
        wcol(wqc, wq_d, 0)
        wcol(wkc, wk_d, 0)
        nc.sync.dma_start(
            xTa[:, 2 * T:4 * T].rearrange("p (cb t) -> p cb t", t=T),
            chunked(xT_d, 2, 2, T))
        nc.sync.dma_start(
            xTa[:, 4 * T:CB * T].rearrange("p (cb t) -> p cb t", t=T),
            chunked(xT_d, 4, 2, T))
        nc.sync.dma_start(bqk[:, 0:CB], col_fold(bqs_d, CB))
        nc.sync.dma_start(bqk[:, CB:2 * CB], col_fold(bk_d, CB))
        # wva before dep: the v feeders gate head 0's PVs, while dep is now
        # consumed one slot later (by the pow) than the scores
        nc.sync.dma_start(wva[:].rearrange("p (cb c) -> p cb c", c=C),
                          chunked(wv_d, 0, CB, C))
        nc.sync.dma_start(bvr[:], bass.AP(tensor=bvh_d.tensor, offset=bvh_d.offset,
                                          ap=[[0, 1], [1, C]]))
        nc.vector.memset(ones1[:], 1.0)
        nc.sync.dma_start(dep_t[0][:], dep_d[0:128, :])
        nc.sync.dma_start(dep_t[1][:], dep_d[128:256, :])
        for cb in range(1, CB):
            wcol(wqc, wq_d, cb)
            wcol(wkc, wk_d, cb)
            nc.sync.dma_start(dep_t[cb + 1][:],
                              dep_d[(cb + 1) * 128:(cb + 2) * 128, :])
        nc.sync.dma_start(dep_t[7][:], dep_d[7 * 128:8 * 128, :])
        nc.sync.dma_start(wgb[:], bcast(wg_d, 2 * C))
        nc.sync.dma_start(nbg_t[:], bcast(nbg_d, 1))

        # ---- stage-A emission units
        def proj_one(dst, w, cb, bcol, eng):
            # full Q^T or K^T chunk through the scores ring (used for qk(0)
            # ahead of head 0 only): kb-outer so matmuls start as DMA chunks
            # land; per-half evictions unblock the first scores early.
            ps = psS.tile([128, T], F32, tag="psS", name="psS")
            for kb in range(CB):
                for tch in range(2):
                    nc.tensor.matmul(
                        ps[:, tch * 512:(tch + 1) * 512],
                        w[cb][:, kb * 128:(kb + 1) * 128],
                        xT[kb][:, tch * 512:(tch + 1) * 512],
                        start=(kb == 0), stop=(kb == CB - 1))
            for tch in range(2):
                eng.tensor_scalar(
                    dst[cb][:, tch * 512:(tch + 1) * 512],
                    ps[:, tch * 512:(tch + 1) * 512], 1.0,
                    bqk[:, bcol:bcol + 1], ALU.mult, ALU.add)

        def qk_half(dst, w, cb, bcol, tch):
            # 512-col half of a Q^T/K^T chunk through the 1-bank psA scratch
            ps = psA.tile([128, 512], F32, tag="psA", name="psA")
            for kb in range(CB):
                nc.tensor.matmul(
                    ps[:], w[cb][:, kb * 128:(kb + 1) * 128],
                    xT[kb][:, tch * 512:(tch + 1) * 512],
                    start=(kb == 0), stop=(kb == CB - 1))
            with tc.high_priority():
                if QK_ACT:
                    nc.scalar.activation(
                        dst[cb][:, tch * 512:(tch + 1) * 512], ps[:],
                        AF.Identity, bias=bqk[:, bcol:bcol + 1], scale=1.0)
                else:
                    nc.vector.tensor_scalar(
                        dst[cb][:, tch * 512:(tch + 1) * 512], ps[:], 1.0,
                        bqk[:, bcol:bcol + 1], ALU.mult, ALU.add)

        def v_half(sb, half):
            # V natural [s part, c free] 512/256-col chunk -> v_aug.  The
            # bias lands via a k=1 ones-row matmul so the eviction is a
            # pure copy (DVE).
            n0, nw = (0, 512) if half == 0 else (512, 256)
            ps = psA.tile([128, 512], F32, tag="psA", name="psA_v")
            for kb in range(CB):
                nc.tensor.matmul(
                    ps[:, 0:nw],
                    xT[kb][:, sb * 128:(sb + 1) * 128],
                    wvs[kb][:, n0:n0 + nw],
                    start=(kb == 0), stop=False)
            nc.tensor.matmul(
                ps[:, 0:nw], ones1[:, sb * 128:(sb + 1) * 128],
                bvr[:, n0:n0 + nw], start=False, stop=True)
            va3 = vaug[sb][:].rearrange("p (h d) -> p h d", d=65)
            h0 = n0 // 64
            ps3 = ps[:, 0:nw].rearrange("p (h d) -> p h d", d=64)
            with tc.high_priority():
                nc.scalar.activation(va3[:, h0:h0 + nw // 64, 0:64], ps3,
                                     AF.Copy)
            if half == 0:
                nc.vector.memset(va3[:, :, 64:65], 1.0)

        # Stage-A feeders are emitted BEFORE their consumers (emission order
        # is semantic order in Tile: a reader emitted before its writer reads
        # garbage), but with the priority counter pushed past every normal
        # instruction, so the scheduler only runs them in engine idle slots.
        from contextlib import contextmanager
        feeder_prio = [1 << 20]

        @contextmanager
        def low_priority():
            orig = tc.cur_priority
            tc.cur_priority = feeder_prio[0]
            try:
                yield
            finally:
                feeder_prio[0] = tc.cur_priority
                tc.cur_priority = orig

        def qk_feed(cb, qk, t):
            return lambda: qk_half((qT, kT)[qk], (wqc, wkc)[qk],
                                   cb, qk * CB + cb, t)
        def v_feed(sb, hf):
            return lambda: v_half(sb, hf)
        # feed_slots[i] = feeder thunks emitted at flat-loop slot i, spread
        # so the PE feeder load is ~flat until head 9.  Deadlines: vaug[sb]
        # half-0 by slot sb+5, qk(cb) by slot 16*cb, vaug half-1 by slot 69.
        feed_slots = {}

        def add_feed(slot, fn):
            feed_slots.setdefault(slot, []).append(fn)

        for sb in range(SB):
            add_feed(sb, v_feed(sb, 0))
        for k, (qk, t) in enumerate((q, t) for q in range(2) for t in range(2)):
            add_feed(8 + 2 * k, qk_feed(1, qk, t))
            add_feed(16 + 4 * k, qk_feed(2, qk, t))
            add_feed(32 + 4 * k, qk_feed(3, qk, t))
            add_feed(47 + 4 * k, qk_feed(4, qk, t))
            add_feed(69 + 2 * k, qk_feed(5, qk, t))
        for sb in range(SB):
            add_feed(60 + sb, v_feed(sb, 1))

        pb = ctx.enter_context(tc.tile_pool(name="pP", bufs=7))
        pdp = ctx.enter_context(tc.tile_pool(name="pdp", bufs=5))
        sdp = ctx.enter_context(tc.tile_pool(name="sdp", bufs=3))
        rp = ctx.enter_context(tc.tile_pool(name="rp", bufs=16))
        cp = ctx.enter_context(tc.tile_pool(name="stageC", bufs=2))
        zp = ctx.enter_context(tc.tile_pool(name="zP", bufs=3))

        # ---- stage B: flat software-pipelined loop over all (h, sb) slots.
        # Skews (in slots): affine +1, PV-b1 +5, PV-b2 +4, psum evict +12.
        # This spreads each head-boundary chain (mul7 -> affine7 -> PV-b2(7)
        # -> recips/evicts -> next head's start=True PVs) across ~5 slots so
        # no in-order engine ever head-of-line-blocks on it.
        NS = H * SB
        pO_l = [None] * NS
        sd_l = [None] * NS
        pD_l = [None] * NS
        ppvh = [None] * H

        def slot_scores(i):
            h, sb = divmod(i, SB)
            hb, hoff = h // 2, (h % 2) * 64
            ps = psS.tile([128, T], F32, tag="psS", name="psS")
            for tch in range(2):
                nc.tensor.matmul(
                    ps[:, tch * 512:(tch + 1) * 512],
                    kT[hb][hoff:hoff + 64, sb * 128:(sb + 1) * 128],
                    qT[hb][hoff:hoff + 64, tch * 512:(tch + 1) * 512],
                    start=True, stop=True)
            pO = pb.tile([128, T], BF16, tag="pO", name="pO", bufs=12)
            nc.scalar.activation(pO[:], ps[:], AF.Exp)
            pO_l[i] = pO
            if B2_ENG.get((h, sb)) == "dve":
                sd = sdp.tile([128, T], BF16, tag="sd", name="sd",
                              bufs=SD_BUFS)
                with tc.high_priority():
                    nc.vector.tensor_mul(sd[:], ps[:], dep_t[sb][:])
                sd_l[i] = sd

        def slot_affine(i):
            # dep-branch exp: Pool slots compute pd = pow(pO, dep) directly
            # (GPSIMD software pow, ~0.4% max err); DVE slots use the
            # Schraudolph int16 affine on sd = s*dep (4x mode).
            h, sb = divmod(i, SB)
            eng = B2_ENG.get((h, sb), "pool")
            if eng == "pool":
                pD = pdp.tile([128, T], BF16, tag="pD", name="pD", bufs=6)
                nc.gpsimd.tensor_tensor(pD[:], pO_l[i][:], dep_t[sb][:],
                                        ALU.pow)
                pD_l[i] = pD[:]
                if dbg2_d is not None and i == 0:
                    nc.sync.dma_start(dbg2_d, pD[:])
            else:
                pD = pdp.tile([128, T], I16, tag="pDi", name="pDi", bufs=4)
                nc.vector.tensor_scalar(pD[:], sd_l[i][:], EXPA, EXPB,
                                        ALU.mult, ALU.add)
                pD_l[i] = pD[:].bitcast(BF16)

        def slot_pv(i, br):
            h, sb = divmod(i, SB)
            if ppvh[h] is None:
                ppvh[h] = [psPV.tile([128, 130 * ntb], F32,
                                     tag=f"ppv{ti}", name=f"ppv{ti}")
                           for ti, (_, ntb) in enumerate(PV_TILES)]
            px = pO_l[i][:] if br == 0 else pD_l[i]
            for tb in range(SB):
                ti, base = PV_OF[tb]
                lhsT = px[:, tb * 128:(tb + 1) * 128]
                nc.tensor.matmul(
                    ppvh[h][ti][:, base + 65 * br:base + 65 * br + 65],
                    lhsT, vaug[sb][:, h * 65:h * 65 + 65],
                    # start=True clears has_written for the WHOLE bank: only
                    # the very first matmul into each psum tile may set it —
                    # that is the b2(sb0) group (emitted at slot h*8+4, one
                    # before b1(sb0)).
                    start=(br == 1 and sb == 0 and base == 0),
                    stop=(sb == SB - 1),
                    skip_group_check=True)

        def evict_head(h):
            # normalize + evict into od[tb] = [osa | dsa] (bf16): one DVE
            # mul per psum tile covering all its t-blocks [p, tb, br, 64]
            for ti, (_, ntb) in enumerate(PV_TILES):
                ppv = ppvh[h][ti]
                rec = rp.tile([128, 8], F32, tag="rec", name="rec")
                den3 = ppv[:].rearrange("p (g d) -> p g d", d=65)[:, :, 64:65]
                nc.vector.reciprocal(
                    rec[:, 0:2 * ntb].rearrange("p (g d) -> p g d", d=1), den3)
                tb0 = sum(n for _, n in PV_TILES[:ti])
                out4 = od_all[:, tb0 * 2 * C:(tb0 + ntb) * 2 * C].rearrange(
                    "p (t b c) -> p t b c", b=2, c=C)[:, :, :, h * 64:(h + 1) * 64]
                in4 = ppv[:].rearrange(
                    "p (t b d) -> p t b d", b=2, d=65)[:, :, :, 0:64]
                recs = rec[:, 0:2 * ntb]
                rec4 = bass.AP(
                    tensor=recs.tensor, offset=recs.offset,
                    ap=[recs.ap[0], [2, ntb], [1, 2], [0, 64]])
                nc.vector.tensor_mul(out4, in4, rec4)

        # ---- stage C: gate + blend in two head-groups. Group 0 (heads 0-8
        # columns) runs during heads 9-11, filling engine slack; only group
        # 1 (3 heads) plus the short gate chain sits in the kernel tail.
        # z-accumulation: g0 on Pool, g1 on DVE (idle in the tail).
        # group column bounds per branch: g0 heads 0-8 (ready after
        # evict(8) @76), g0b heads 9-10 (after evict(10) @92), g1 head 11
        GB = (0, 9 * D, 11 * D, C)
        z_parts = [[None, None, None] for _ in range(SB)]
        diffs = [None] * SB

        def stageC_grp(tb, g):
            c0, c1 = GB[g], GB[g + 1]
            w = c1 - c0
            if g == 0:
                diffs[tb] = cp.tile([128, C], BF16, tag="diff", name="diff",
                                    bufs=SB)
            od3 = od[tb].rearrange("p (b c) -> p b c", b=2)[:, :, c0:c1]
            tod = cp.tile([128, 2 * w], BF16, tag=f"tod{g}", name="tod")
            tod3 = tod[:].rearrange("p (b c) -> p b c", b=2)
            nc.scalar.activation(tod3, od3, AF.Tanh)
            scr = cp.tile([128, 2 * w], BF16, tag=f"scr{g}", name="scr", bufs=1)
            z = zp.tile([128, 1], F32, tag=f"z{g}", name="z", bufs=SB + 1)
            wg3 = wgb[:].rearrange("p (b c) -> p b c", b=2)[:, :, c0:c1]
            zeng = nc.gpsimd if (g == 0 and Z_G0_POOL(tb)) else nc.vector
            zeng.scalar_tensor_tensor(
                out=scr[:].rearrange("p (b c) -> p b c", b=2), in0=tod3,
                scalar=0.0, in1=wg3, op0=ALU.bypass, op1=ALU.mult,
                accum_out=z[:])
            z_parts[tb][g] = z
            nc.vector.tensor_sub(diffs[tb][:, c0:c1], od3[:, 0, :], od3[:, 1, :])

        def stageC_back(tb):
            zt0 = zp.tile([128, 1], F32, tag="zt0", name="zt0")
            nc.vector.tensor_add(zt0[:], z_parts[tb][0][:], z_parts[tb][1][:])
            zt = zp.tile([128, 1], F32, tag="zt", name="zt")
            nc.vector.tensor_add(zt[:], zt0[:], z_parts[tb][2][:])
            g = zp.tile([128, 1], F32, tag="g", name="g")
            # g = sigmoid(z + bg) in one ACT op (bias = +bg, so negate nbg)
            nc.scalar.activation(g[:], zt[:], AF.Sigmoid, bias=nbg_t[:],
                                 scale=1.0)
            outt = cp.tile([128, C], BF16, tag="outt", name="outt", bufs=4)
            # all-bf16 SBUF stst (no accum) hits DVE 4x mode: ~260ns
            nc.vector.scalar_tensor_tensor(
                out=outt[:], in0=diffs[tb][:], scalar=g[:],
                in1=od[tb][:, C:2 * C], op0=ALU.mult, op1=ALU.add)
            nc.sync.dma_start(out_d[tb * 128:(tb + 1) * 128, :], outt[:])

        # ---- emission: qk(0) through the scores ring (evictions interleaved
        # Qh0, Kh0, Qh1, Kh1 so the first scores matmul unblocks earliest),
        # then all heads with a low-priority feeder batch before each
        ps_qk = [psS.tile([128, T], F32, tag="psS", name=f"psQK{i}")
                 for i in range(2)]
        for kb in range(CB):
            for tch in range(2):
                for i, w in enumerate((wqc, wkc)):
                    nc.tensor.matmul(
                        ps_qk[i][:, tch * 512:(tch + 1) * 512],
                        w[0][:, kb * 128:(kb + 1) * 128],
                        xT[kb][:, tch * 512:(tch + 1) * 512],
                        start=(kb == 0), stop=(kb == CB - 1))
        for tch in range(2):
            # Q half on DVE in parallel with K half on the (idle) ACT engine
            nc.vector.tensor_scalar(
                qT[0][:, tch * 512:(tch + 1) * 512],
                ps_qk[0][:, tch * 512:(tch + 1) * 512], 1.0,
                bqk[:, 0:1], ALU.mult, ALU.add)
            nc.scalar.activation(
                kT[0][:, tch * 512:(tch + 1) * 512],
                ps_qk[1][:, tch * 512:(tch + 1) * 512],
                AF.Identity, bias=bqk[:, CB:CB + 1], scale=1.0)
        for i in range(NS + 13):
            if i < NS:
                slot_scores(i)
            if i in feed_slots:
                with low_priority():
                    for f in feed_slots[i]:
                        f()
            if i >= 77 and (i - 77) % 2 == 0 and (i - 77) // 2 < SB:
                # heads 0-8 od columns complete (evict(8) at slot 76)
                with low_priority():
                    stageC_grp((i - 77) // 2, 0)
            if i >= 93 and i - 93 < SB:
                # heads 9-10 od columns complete (evict(10) at slot 92)
                with low_priority():
                    stageC_grp(i - 93, 1)
            if 1 <= i <= NS and i - 1 < NS:
                slot_affine(i - 1)
            # slot h*8+4 carries, in order: b1(h-1, sb7) (completes head
            # h-1's accumulation), evict(h-1) (reads the full bank), then
            # b2(h, sb0) with start=True (clobbers the bank for head h)
            if i >= 5 and i - 5 < NS:
                slot_pv(i - 5, 0)
            if i >= 12 and (i - 12) % SB == 0 and (i - 12) // SB < H:
                evict_head((i - 12) // SB)
            if i >= 4 and i - 4 < NS:
                slot_pv(i - 4, 1)

        for tb in range(SB):
            stageC_grp(tb, 2)
            if tb >= 1:
                stageC_back(tb - 1)
        stageC_back(SB - 1)
        if debug:
            nc.sync.dma_start(dbg_d, od_all[:])

    nc.finalize()
    return nc


_CACHE = {}


def _prep_in_maps(hidden_states, dependency_matrix, Wq, bq, Wk, bk, Wv, bv, Wg, bg):
    hs = np.asarray(hidden_states, dtype=np.float32)
    dep = np.asarray(dependency_matrix, dtype=np.float32)
    bf = ml_dtypes.bfloat16
    shared = {
        # 1/sqrt(D) score scale folded into Wq (exact: power of two)
        "wq": np.ascontiguousarray((np.asarray(Wq, np.float32) * 0.125).T).astype(bf),
        "wk": np.ascontiguousarray(np.asarray(Wk, np.float32).T).astype(bf),
        "wv": np.ascontiguousarray(np.asarray(Wv, np.float32).T).astype(bf),
        "bqs": (np.asarray(bq, np.float32) * 0.125).reshape(C),
        "bk": np.asarray(bk, np.float32).reshape(C),
        "bvh": np.ascontiguousarray(np.asarray(bv, np.float32).reshape(C)).astype(bf),
        "wg": np.ascontiguousarray(np.asarray(Wg, np.float32).reshape(2 * C)).astype(bf),
        # "nbg" now carries +bg (Sigmoid bias), name kept for dram binding
        "nbg": np.asarray(bg, np.float32).reshape(1),
    }
    in_maps = []
    for b in range(B):
        m = dict(shared)
        m["xT"] = np.ascontiguousarray(hs[b].T).astype(bf)
        m["dep"] = np.ascontiguousarray(dep[b].T).astype(bf)
        in_maps.append(m)
    return in_maps


def kernel(**inputs):
    from concourse.bass_utils import run_bass_kernel_spmd
    if "nc" not in _CACHE:
        _CACHE["nc"] = build_nc()
    nc = _CACHE["nc"]
    in_maps = _prep_in_maps(**inputs)
    res = run_bass_kernel_spmd(nc, in_maps, core_ids=list(range(NCORES)))
    out = np.stack([res.results[i]["out"] for i in range(NCORES)], axis=0)
    return out.astype(np.float32)



# revision 64
# speedup vs baseline: 1.2059x; 1.0049x over previous
"""DependencyBertSelfAttention Trainium2 kernel (v5).

Sharding: batch B=8 -> one batch element per NeuronCore (8 cores, SPMD).
Per core (full T=1024, C=768, H=12 heads, D=64):

  Stage B runs as ONE flat software-pipelined loop over all 96 (head,
  s-block) slots with fixed skews: scores+exp at slot i, branch-2 exp at
  i+1, PV-b1 at i+5, PV-b2 at i+4, psum eviction at i+12.  Each
  head-boundary chain (last scores -> pow -> PV stop -> recip/evict ->
  next head's start=True PVs) therefore spreads over ~5 slots and no
  in-order engine head-of-line-blocks on it.

  Branch 1: pO = exp(S^T) on ACT (the only exp-capable engine).
  Branch 2: pd = pow(pO, dep^T) — a single GPSIMD tensor_tensor op
  (software pow, ~0.4% max err) for most slots; a quarter of the slots
  instead run sd = S*dep (DVE) + Schraudolph int16 affine
  round(sd*2^7*log2e + (127*2^7-5.5)) whose int16 bits ARE the bf16
  pattern of exp(sd) (DVE 4x mode), balancing Pool vs DVE load.

  PSUM (8 banks): scores ring psS [128,1024]x2 (4), feeder scratch psA
  [128,512] (1), PV accumulators 3 ([osa|dsa] 65-col pairs per t-block,
  packed 390/390/260; 65th vaug column of ones = softmax denominator).
  PV-weight matmuls must keep out <= 512 f32 (one bank) per instruction.

  Projections: qk(0) through the scores ring up front; remaining Q/K/V
  512-col chunks are low-priority feeders through psA spread over slots
  0-76 (deadline: qk(cb) by slot 16cb, v half-1 by slot 64). V bias is
  added via a k=1 ones-row matmul so the eviction is a pure copy.

  Normalization: one DVE mul per psum tile [p, tb, branch, 64] using
  per-(tb,branch) reciprocals; od packed as one [p, tb, branch, C] tile.

  Stage C in three column groups so only head 11's columns sit in the
  kernel tail: g0 (heads 0-8) from slot 77, g0b (heads 9-10) from slot
  93, g1 (head 11) after the flat loop.  Per (tb, group): tanh (ACT),
  z-accum stst (DVE), diff (DVE 2x); then zt = z0+z0b+z1,
  g = Sigmoid(zt+bg) (ACT), blend stst, DMA out.

No inter-core communication: each core's batch element is independent.
"""
import sys

sys.path.insert(0, "/opt/trn_rl_repo")

import numpy as np
import ml_dtypes
from contextlib import ExitStack

import concourse.bass as bass
import concourse.tile as tile
from concourse import bacc, mybir

B, T, C, H, D = 8, 1024, 768, 12, 64
CB = C // 128   # 6 channel partition-blocks
SB = T // 128   # 8 s/t blocks
NCORES = 8

F32 = mybir.dt.float32
BF16 = mybir.dt.bfloat16
I16 = mybir.dt.int16
AF = mybir.ActivationFunctionType
ALU = mybir.AluOpType

# Schraudolph exp in bf16 bit-space: round(x*2^7*log2e + (127*2^7 - 5.5))
# as int16 == bf16 bit pattern of ~exp(x); the one-sided mantissa-sawtooth
# error cancels in the softmax ratio (measured 8.7e-3 end-to-end rel err).
EXPA = float((1 << 7) * 1.4426950408889634)
EXPB = float(127 * (1 << 7) - 5.5)

# engine assignment knobs (tuned against TimelineSim)
def _b2_eng(h, sb):
    # "pool": pd = pow(pO, dep) — one GPSIMD op, no dep-mul at all.
    # "dve": sd = ps*dep (DVE) + Schraudolph int16 affine (DVE 4x) — used
    # for a fraction of slots to balance Pool vs DVE.
    return "dve" if (h * SB + sb) % 4 == 3 else "pool"


B2_ENG = {(h, sb): _b2_eng(h, sb) for h in range(H) for sb in range(SB)}
Z_G0_POOL = lambda tb: False
QK_ACT = False
SD_BUFS = 4

# PV psum packing: tile index and column base for each t-block
PV_TILES = ((0, 3), (1, 3), (2, 2))          # (tile idx, n t-blocks)
PV_OF = {}
for _ti, (_i, _n) in enumerate(PV_TILES):
    for _k in range(_n):
        PV_OF[sum(n for _, n in PV_TILES[:_ti]) + _k] = (_ti, _k * 130)


def build_nc(debug=False, repeat=1):
    nc = bacc.Bacc("TRN2", target_bir_lowering=False, debug=False,
                   num_devices=NCORES)

    xT_d = nc.dram_tensor("xT", [C, T], BF16, kind="ExternalInput").ap()
    wq_d = nc.dram_tensor("wq", [C, C], BF16, kind="ExternalInput").ap()
    wk_d = nc.dram_tensor("wk", [C, C], BF16, kind="ExternalInput").ap()
    wv_d = nc.dram_tensor("wv", [C, C], BF16, kind="ExternalInput").ap()
    bqs_d = nc.dram_tensor("bqs", [C], F32, kind="ExternalInput").ap()
    bk_d = nc.dram_tensor("bk", [C], F32, kind="ExternalInput").ap()
    bvh_d = nc.dram_tensor("bvh", [C], BF16, kind="ExternalInput").ap()
    dep_d = nc.dram_tensor("dep", [T, T], BF16, kind="ExternalInput").ap()
    wg_d = nc.dram_tensor("wg", [2 * C], BF16, kind="ExternalInput").ap()
    nbg_d = nc.dram_tensor("nbg", [1], F32, kind="ExternalInput").ap()
    out_d = nc.dram_tensor("out", [T, C], BF16, kind="ExternalOutput").ap()
    dbg_d = (nc.dram_tensor("dbg", [128, SB * 2 * C], BF16,
                            kind="ExternalOutput").ap() if debug else None)
    dbg2_d = (nc.dram_tensor("dbg2", [128, T], BF16,
                             kind="ExternalOutput").ap() if debug else None)

    def bcast(src_ap, n_free):
        return bass.AP(tensor=src_ap.tensor, offset=src_ap.offset,
                       ap=[[0, 128], [1, n_free]])

    def chunked(src_ap, cb0, ncb, width):
        # dram [C, width] rows cb0*128.. viewed as [p, cb, w] for one DMA
        return bass.AP(tensor=src_ap.tensor, offset=src_ap.offset + cb0 * 128 * width,
                       ap=[[width, 128], [128 * width, ncb], [1, width]])

    def col_chunk(src_ap, cb):
        # dram [C, C] columns cb*128..: [p, kb, j] = src[kb*128+p, cb*128+j]
        return bass.AP(tensor=src_ap.tensor, offset=src_ap.offset + cb * 128,
                       ap=[[C, 128], [128 * C, CB], [1, 128]])

    def col_fold(src_ap, ncb):
        # dram [C] -> [128, ncb]: partition p, col cb = src[cb*128 + p]
        return bass.AP(tensor=src_ap.tensor, offset=src_ap.offset,
                       ap=[[1, 128], [128, ncb]])

    with tile.TileContext(nc, pool_alloc_mode="queue") as tc, ExitStack() as ctx:
      persist = ctx.enter_context(tc.tile_pool(name="persist", bufs=1))
      psS = ctx.enter_context(tc.tile_pool(name="psS", bufs=2, space="PSUM"))
      psA = ctx.enter_context(tc.tile_pool(name="psA", bufs=1, space="PSUM"))
      psPV = ctx.enter_context(tc.tile_pool(name="psPV", bufs=1, space="PSUM"))
      for _rep in range(repeat):
        # ---- persistent tiles
        xTa = persist.tile([128, CB * T], BF16, tag="xTa", name="xTa")
        # weights as per-output-cb column tiles: wqc[cb][p, kb*128+j] =
        # Wq.T[kb*128+p, cb*128+j] — head 0 needs only cb=0 (196KB, not 1.2MB)
        wqc = [persist.tile([128, CB * 128], BF16, tag=f"wqc{i}", name=f"wqc{i}")
               for i in range(CB)]
        wkc = [persist.tile([128, CB * 128], BF16, tag=f"wkc{i}", name=f"wkc{i}")
               for i in range(CB)]
        wva = persist.tile([128, CB * C], BF16, tag="wva", name="wva")
        xT = [xTa[:, i * T:(i + 1) * T] for i in range(CB)]
        wvs = [wva[:, i * C:(i + 1) * C] for i in range(CB)]
        qT = [persist.tile([128, T], BF16, tag=f"qT{i}", name=f"qT{i}") for i in range(CB)]
        kT = [persist.tile([128, T], BF16, tag=f"kT{i}", name=f"kT{i}") for i in range(CB)]
        vaug = [persist.tile([128, H * 65], BF16, tag=f"vaug{i}", name=f"vaug{i}") for i in range(SB)]
        dep_t = [persist.tile([128, T], BF16, tag=f"dep{i}", name=f"dep{i}") for i in range(SB)]
        # od packed as one tile [p, tb, branch, C] so per-head normalization
        # evictions can cover all t-blocks of a psum tile in one instruction
        od_all = persist.tile([128, SB * 2 * C], BF16, tag="od", name="od")
        od = [od_all[:, i * 2 * C:(i + 1) * 2 * C] for i in range(SB)]
        bqk = persist.tile([128, 2 * CB], F32, tag="bqk", name="bqk")
        ones1 = persist.tile([1, T], BF16, tag="ones1", name="ones1")
        bvr = persist.tile([1, C], BF16, tag="bvr", name="bvr")
        wgb = persist.tile([128, 2 * C], BF16, tag="wgb", name="wgb")
        nbg_t = persist.tile([128, 1], F32, tag="nbg", name="nbg")

        # warm the ACT exp/tanh table set at t=0 so the first real exp
        # doesn't pay the ~2.7us table load mid-ramp; keep PE continuously
        # busy with dummy matmuls so its p-state ramps to full clock before
        # the first projection matmuls arrive (~6us in)
        with tc.high_priority():
            warm = persist.tile([128, 1], F32, tag="warm", name="warm")
            nc.vector.memset(warm[:], 0.0)
            nc.scalar.activation(warm[:], warm[:], AF.Exp)
            zz = persist.tile([128, 512], BF16, tag="zz", name="zz")
            nc.vector.memset(zz[:], 0.0)
            wz = persist.tile([128, 1], BF16, tag="wz", name="wz")
            nc.vector.memset(wz[:], 0.0)
            for _ in range(8):
                pwarm = psS.tile([128, T], F32, tag="psS", name="pwarm")
                nc.tensor.matmul(pwarm[0:1, 0:512], wz[:], zz[:],
                                 start=True, stop=True)

        # ---- DMAs on the sync queue: the serialized DMA engines process
        # transfers in exactly this order, so startup-critical data (x, the
        # cb=0 weight columns) comes first.
        hc = CB // 2
        def wcol(dst, src, cb):
            nc.sync.dma_start(
                dst[cb][:].rearrange("p (kb j) -> p kb j", j=128),
                col_chunk(src, cb))
        nc.sync.dma_start(
            xTa[:, 0:2 * T].rearrange("p (cb t) -> p cb t", t=T),
            chunked(xT_d, 0, 2, T))
        wcol(wqc, wq_d, 0)
        wcol(wkc, wk_d, 0)
        nc.sync.dma_start(
            xTa[:, 2 * T:4 * T].rearrange("p (cb t) -> p cb t", t=T),
            chunked(xT_d, 2, 2, T))
        nc.sync.dma_start(
            xTa[:, 4 * T:CB * T].rearrange("p (cb t) -> p cb t", t=T),
            chunked(xT_d, 4, 2, T))
        nc.sync.dma_start(bqk[:, 0:CB], col_fold(bqs_d, CB))
        nc.sync.dma_start(bqk[:, CB:2 * CB], col_fold(bk_d, CB))
        # wva before dep: the v feeders gate head 0's PVs, while dep is now
        # consumed one slot later (by the pow) than the scores
        nc.sync.dma_start(wva[:].rearrange("p (cb c) -> p cb c", c=C),
                          chunked(wv_d, 0, CB, C))
        nc.sync.dma_start(bvr[:], bass.AP(tensor=bvh_d.tensor, offset=bvh_d.offset,
                                          ap=[[0, 1], [1, C]]))
        nc.vector.memset(ones1[:], 1.0)
        nc.sync.dma_start(dep_t[0][:], dep_d[0:128, :])
        nc.sync.dma_start(dep_t[1][:], dep_d[128:256, :])
        for cb in range(1, CB):
            wcol(wqc, wq_d, cb)
            wcol(wkc, wk_d, cb)
            nc.sync.dma_start(dep_t[cb + 1][:],
                              dep_d[(cb + 1) * 128:(cb + 2) * 128, :])
        nc.sync.dma_start(dep_t[7][:], dep_d[7 * 128:8 * 128, :])
        nc.sync.dma_start(wgb[:], bcast(wg_d, 2 * C))
        nc.sync.dma_start(nbg_t[:], bcast(nbg_d, 1))

        # ---- stage-A emission units
        def proj_one(dst, w, cb, bcol, eng):
            # full Q^T or K^T chunk through the scores ring (used for qk(0)
            # ahead of head 0 only): kb-outer so matmuls start as DMA chunks
            # land; per-half evictions unblock the first scores early.
            ps = psS.tile([128, T], F32, tag="psS", name="psS")
            for kb in range(CB):
                for tch in range(2):
                    nc.tensor.matmul(
                        ps[:, tch * 512:(tch + 1) * 512],
                        w[cb][:, kb * 128:(kb + 1) * 128],
                        xT[kb][:, tch * 512:(tch + 1) * 512],
                        start=(kb == 0), stop=(kb == CB - 1))
            for tch in range(2):
                eng.tensor_scalar(
                    dst[cb][:, tch * 512:(tch + 1) * 512],
                    ps[:, tch * 512:(tch + 1) * 512], 1.0,
                    bqk[:, bcol:bcol + 1], ALU.mult, ALU.add)

        def qk_half(dst, w, cb, bcol, tch):
            # 512-col half of a Q^T/K^T chunk through the 1-bank psA scratch
            ps = psA.tile([128, 512], F32, tag="psA", name="psA")
            for kb in range(CB):
                nc.tensor.matmul(
                    ps[:], w[cb][:, kb * 128:(kb + 1) * 128],
                    xT[kb][:, tch * 512:(tch + 1) * 512],
                    start=(kb == 0), stop=(kb == CB - 1))
            with tc.high_priority():
                if QK_ACT:
                    nc.scalar.activation(
                        dst[cb][:, tch * 512:(tch + 1) * 512], ps[:],
                        AF.Identity, bias=bqk[:, bcol:bcol + 1], scale=1.0)
                else:
                    nc.vector.tensor_scalar(
                        dst[cb][:, tch * 512:(tch + 1) * 512], ps[:], 1.0,
                        bqk[:, bcol:bcol + 1], ALU.mult, ALU.add)

        def v_half(sb, half):
            # V natural [s part, c free] 512/256-col chunk -> v_aug.  The
            # bias lands via a k=1 ones-row matmul so the eviction is a
            # pure copy (DVE).
            n0, nw = (0, 512) if half == 0 else (512, 256)
            ps = psA.tile([128, 512], F32, tag="psA", name="psA_v")
            for kb in range(CB):
                nc.tensor.matmul(
                    ps[:, 0:nw],
                    xT[kb][:, sb * 128:(sb + 1) * 128],
                    wvs[kb][:, n0:n0 + nw],
                    start=(kb == 0), stop=False)
            nc.tensor.matmul(
                ps[:, 0:nw], ones1[:, sb * 128:(sb + 1) * 128],
                bvr[:, n0:n0 + nw], start=False, stop=True)
            va3 = vaug[sb][:].rearrange("p (h d) -> p h d", d=65)
            h0 = n0 // 64
            ps3 = ps[:, 0:nw].rearrange("p (h d) -> p h d", d=64)
            with tc.high_priority():
                nc.scalar.activation(va3[:, h0:h0 + nw // 64, 0:64], ps3,
                                     AF.Copy)
            if half == 0:
                nc.vector.memset(va3[:, :, 64:65], 1.0)

        # Stage-A feeders are emitted BEFORE their consumers (emission order
        # is semantic order in Tile: a reader emitted before its writer reads
        # garbage), but with the priority counter pushed past every normal
        # instruction, so the scheduler only runs them in engine idle slots.
        from contextlib import contextmanager
        feeder_prio = [1 << 20]

        @contextmanager
        def low_priority():
            orig = tc.cur_priority
            tc.cur_priority = feeder_prio[0]
            try:
                yield
            finally:
                feeder_prio[0] = tc.cur_priority
                tc.cur_priority = orig

        def qk_feed(cb, qk, t):
            return lambda: qk_half((qT, kT)[qk], (wqc, wkc)[qk],
                                   cb, qk * CB + cb, t)
        def v_feed(sb, hf):
            return lambda: v_half(sb, hf)
        # feed_slots[i] = feeder thunks emitted at flat-loop slot i, spread
        # so the PE feeder load is ~flat until head 9.  Deadlines: vaug[sb]
        # half-0 by slot sb+5, qk(cb) by slot 16*cb, vaug half-1 by slot 69.
        feed_slots = {}

        def add_feed(slot, fn):
            feed_slots.setdefault(slot, []).append(fn)

        for sb in range(SB):
            add_feed(sb, v_feed(sb, 0))
        for k, (qk, t) in enumerate((q, t) for q in range(2) for t in range(2)):
            add_feed(8 + 2 * k, qk_feed(1, qk, t))
            add_feed(16 + 4 * k, qk_feed(2, qk, t))
            add_feed(32 + 4 * k, qk_feed(3, qk, t))
            add_feed(47 + 4 * k, qk_feed(4, qk, t))
            add_feed(69 + 2 * k, qk_feed(5, qk, t))
        for sb in range(SB):
            add_feed(60 + sb, v_feed(sb, 1))

        pb = ctx.enter_context(tc.tile_pool(name="pP", bufs=7))
        pdp = ctx.enter_context(tc.tile_pool(name="pdp", bufs=5))
        sdp = ctx.enter_context(tc.tile_pool(name="sdp", bufs=3))
        rp = ctx.enter_context(tc.tile_pool(name="rp", bufs=16))
        cp = ctx.enter_context(tc.tile_pool(name="stageC", bufs=2))
        zp = ctx.enter_context(tc.tile_pool(name="zP", bufs=3))

        # ---- stage B: flat software-pipelined loop over all (h, sb) slots.
        # Skews (in slots): affine +1, PV-b1 +5, PV-b2 +4, psum evict +12.
        # This spreads each head-boundary chain (mul7 -> affine7 -> PV-b2(7)
        # -> recips/evicts -> next head's start=True PVs) across ~5 slots so
        # no in-order engine ever head-of-line-blocks on it.
        NS = H * SB
        pO_l = [None] * NS
        sd_l = [None] * NS
        pD_l = [None] * NS
        ppvh = [None] * H

        def slot_scores(i):
            h, sb = divmod(i, SB)
            hb, hoff = h // 2, (h % 2) * 64
            ps = psS.tile([128, T], F32, tag="psS", name="psS")
            for tch in range(2):
                nc.tensor.matmul(
                    ps[:, tch * 512:(tch + 1) * 512],
                    kT[hb][hoff:hoff + 64, sb * 128:(sb + 1) * 128],
                    qT[hb][hoff:hoff + 64, tch * 512:(tch + 1) * 512],
                    start=True, stop=True)
            pO = pb.tile([128, T], BF16, tag="pO", name="pO", bufs=12)
            nc.scalar.activation(pO[:], ps[:], AF.Exp)
            pO_l[i] = pO
            if B2_ENG.get((h, sb)) == "dve":
                sd = sdp.tile([128, T], BF16, tag="sd", name="sd",
                              bufs=SD_BUFS)
                with tc.high_priority():
                    nc.vector.tensor_mul(sd[:], ps[:], dep_t[sb][:])
                sd_l[i] = sd

        def slot_affine(i):
            # dep-branch exp: Pool slots compute pd = pow(pO, dep) directly
            # (GPSIMD software pow, ~0.4% max err); DVE slots use the
            # Schraudolph int16 affine on sd = s*dep (4x mode).
            h, sb = divmod(i, SB)
            eng = B2_ENG.get((h, sb), "pool")
            if eng == "pool":
                pD = pdp.tile([128, T], BF16, tag="pD", name="pD", bufs=6)
                nc.gpsimd.tensor_tensor(pD[:], pO_l[i][:], dep_t[sb][:],
                                        ALU.pow)
                pD_l[i] = pD[:]
                if dbg2_d is not None and i == 0:
                    nc.sync.dma_start(dbg2_d, pD[:])
            else:
                pD = pdp.tile([128, T], I16, tag="pDi", name="pDi", bufs=4)
                nc.vector.tensor_scalar(pD[:], sd_l[i][:], EXPA, EXPB,
                                        ALU.mult, ALU.add)
                pD_l[i] = pD[:].bitcast(BF16)

        def slot_pv(i, br):
            h, sb = divmod(i, SB)
            if ppvh[h] is None:
                ppvh[h] = [psPV.tile([128, 130 * ntb], F32,
                                     tag=f"ppv{ti}", name=f"ppv{ti}")
                           for ti, (_, ntb) in enumerate(PV_TILES)]
            px = pO_l[i][:] if br == 0 else pD_l[i]
            for tb in range(SB):
                ti, base = PV_OF[tb]
                lhsT = px[:, tb * 128:(tb + 1) * 128]
                nc.tensor.matmul(
                    ppvh[h][ti][:, base + 65 * br:base + 65 * br + 65],
                    lhsT, vaug[sb][:, h * 65:h * 65 + 65],
                    # start=True clears has_written for the WHOLE bank: only
                    # the very first matmul into each psum tile may set it —
                    # that is the b2(sb0) group (emitted at slot h*8+4, one
                    # before b1(sb0)).
                    start=(br == 1 and sb == 0 and base == 0),
                    stop=(sb == SB - 1),
                    skip_group_check=True)

        def evict_head(h):
            # normalize + evict into od[tb] = [osa | dsa] (bf16): one DVE
            # mul per psum tile covering all its t-blocks [p, tb, br, 64]
            for ti, (_, ntb) in enumerate(PV_TILES):
                ppv = ppvh[h][ti]
                rec = rp.tile([128, 8], F32, tag="rec", name="rec")
                den3 = ppv[:].rearrange("p (g d) -> p g d", d=65)[:, :, 64:65]
                nc.vector.reciprocal(
                    rec[:, 0:2 * ntb].rearrange("p (g d) -> p g d", d=1), den3)
                tb0 = sum(n for _, n in PV_TILES[:ti])
                out4 = od_all[:, tb0 * 2 * C:(tb0 + ntb) * 2 * C].rearrange(
                    "p (t b c) -> p t b c", b=2, c=C)[:, :, :, h * 64:(h + 1) * 64]
                in4 = ppv[:].rearrange(
                    "p (t b d) -> p t b d", b=2, d=65)[:, :, :, 0:64]
                recs = rec[:, 0:2 * ntb]
                rec4 = bass.AP(
                    tensor=recs.tensor, offset=recs.offset,
                    ap=[recs.ap[0], [2, ntb], [1, 2], [0, 64]])
                nc.vector.tensor_mul(out4, in4, rec4)

        # ---- stage C: gate + blend in two head-groups. Group 0 (heads 0-8
        # columns) runs during heads 9-11, filling engine slack; only group
        # 1 (3 heads) plus the short gate chain sits in the kernel tail.
        # z-accumulation: g0 on Pool, g1 on DVE (idle in the tail).
        # group column bounds per branch: g0 heads 0-8 (ready after
        # evict(8) @76), g0b heads 9-10 (after evict(10) @92), g1 head 11
        GB = (0, 9 * D, 11 * D, C)
        z_parts = [[None, None, None] for _ in range(SB)]
        diffs = [None] * SB

        def stageC_grp(tb, g):
            c0, c1 = GB[g], GB[g + 1]
            w = c1 - c0
            if g == 0:
                diffs[tb] = cp.tile([128, C], BF16, tag="diff", name="diff",
                                    bufs=SB)
            od3 = od[tb].rearrange("p (b c) -> p b c", b=2)[:, :, c0:c1]
            tod = cp.tile([128, 2 * w], BF16, tag=f"tod{g}", name="tod")
            tod3 = tod[:].rearrange("p (b c) -> p b c", b=2)
            nc.scalar.activation(tod3, od3, AF.Tanh)
            scr = cp.tile([128, 2 * w], BF16, tag=f"scr{g}", name="scr", bufs=1)
            z = zp.tile([128, 1], F32, tag=f"z{g}", name="z", bufs=SB + 1)
            wg3 = wgb[:].rearrange("p (b c) -> p b c", b=2)[:, :, c0:c1]
            zeng = nc.gpsimd if (g == 0 and Z_G0_POOL(tb)) else nc.vector
            zeng.scalar_tensor_tensor(
                out=scr[:].rearrange("p (b c) -> p b c", b=2), in0=tod3,
                scalar=0.0, in1=wg3, op0=ALU.bypass, op1=ALU.mult,
                accum_out=z[:])
            z_parts[tb][g] = z
            nc.vector.tensor_sub(diffs[tb][:, c0:c1], od3[:, 0, :], od3[:, 1, :])

        def stageC_back(tb):
            zt0 = zp.tile([128, 1], F32, tag="zt0", name="zt0")
            nc.vector.tensor_add(zt0[:], z_parts[tb][0][:], z_parts[tb][1][:])
            zt = zp.tile([128, 1], F32, tag="zt", name="zt")
            nc.vector.tensor_add(zt[:], zt0[:], z_parts[tb][2][:])
            g = zp.tile([128, 1], F32, tag="g", name="g")
            # g = sigmoid(z + bg) in one ACT op (bias = +bg, so negate nbg)
            nc.scalar.activation(g[:], zt[:], AF.Sigmoid, bias=nbg_t[:],
                                 scale=1.0)
            outt = cp.tile([128, C], BF16, tag="outt", name="outt", bufs=4)
            # t1 = diff*g via plain tensor_scalar (DVE 4x: stst would be 1x)
            t1 = cp.tile([128, C], BF16, tag="t1", name="t1", bufs=2)
            nc.vector.tensor_scalar(t1[:], diffs[tb][:], g[:], None, ALU.mult)
            nc.vector.tensor_add(outt[:], t1[:], od[tb][:, C:2 * C])
            nc.sync.dma_start(out_d[tb * 128:(tb + 1) * 128, :], outt[:])

        # ---- emission: qk(0) through the scores ring (evictions interleaved
        # Qh0, Kh0, Qh1, Kh1 so the first scores matmul unblocks earliest),
        # then all heads with a low-priority feeder batch before each
        ps_qk = [psS.tile([128, T], F32, tag="psS", name=f"psQK{i}")
                 for i in range(2)]
        for kb in range(CB):
            for tch in range(2):
                for i, w in enumerate((wqc, wkc)):
                    nc.tensor.matmul(
                        ps_qk[i][:, tch * 512:(tch + 1) * 512],
                        w[0][:, kb * 128:(kb + 1) * 128],
                        xT[kb][:, tch * 512:(tch + 1) * 512],
                        start=(kb == 0), stop=(kb == CB - 1))
        for tch in range(2):
            # Q half on DVE in parallel with K half on the (idle) ACT engine
            nc.vector.tensor_scalar(
                qT[0][:, tch * 512:(tch + 1) * 512],
                ps_qk[0][:, tch * 512:(tch + 1) * 512], 1.0,
                bqk[:, 0:1], ALU.mult, ALU.add)
            nc.scalar.activation(
                kT[0][:, tch * 512:(tch + 1) * 512],
                ps_qk[1][:, tch * 512:(tch + 1) * 512],
                AF.Identity, bias=bqk[:, CB:CB + 1], scale=1.0)
        for i in range(NS + 13):
            if i < NS:
                slot_scores(i)
            if i in feed_slots:
                with low_priority():
                    for f in feed_slots[i]:
                        f()
            if i >= 77 and (i - 77) % 2 == 0 and (i - 77) // 2 < SB:
                # heads 0-8 od columns complete (evict(8) at slot 76)
                with low_priority():
                    stageC_grp((i - 77) // 2, 0)
            if i >= 93 and i - 93 < SB:
                # heads 9-10 od columns complete (evict(10) at slot 92)
                with low_priority():
                    stageC_grp(i - 93, 1)
            if 1 <= i <= NS and i - 1 < NS:
                slot_affine(i - 1)
            # slot h*8+4 carries, in order: b1(h-1, sb7) (completes head
            # h-1's accumulation), evict(h-1) (reads the full bank), then
            # b2(h, sb0) with start=True (clobbers the bank for head h)
            if i >= 5 and i - 5 < NS:
                slot_pv(i - 5, 0)
            if i >= 12 and (i - 12) % SB == 0 and (i - 12) // SB < H:
                evict_head((i - 12) // SB)
            if i >= 4 and i - 4 < NS:
                slot_pv(i - 4, 1)

        for tb in range(SB):
            stageC_grp(tb, 2)
            if tb >= 1:
                stageC_back(tb - 1)
        stageC_back(SB - 1)
        if debug:
            nc.sync.dma_start(dbg_d, od_all[:])

    nc.finalize()
    return nc


_CACHE = {}


def _prep_in_maps(hidden_states, dependency_matrix, Wq, bq, Wk, bk, Wv, bv, Wg, bg):
    hs = np.asarray(hidden_states, dtype=np.float32)
    dep = np.asarray(dependency_matrix, dtype=np.float32)
    bf = ml_dtypes.bfloat16
    shared = {
        # 1/sqrt(D) score scale folded into Wq (exact: power of two)
        "wq": np.ascontiguousarray((np.asarray(Wq, np.float32) * 0.125).T).astype(bf),
        "wk": np.ascontiguousarray(np.asarray(Wk, np.float32).T).astype(bf),
        "wv": np.ascontiguousarray(np.asarray(Wv, np.float32).T).astype(bf),
        "bqs": (np.asarray(bq, np.float32) * 0.125).reshape(C),
        "bk": np.asarray(bk, np.float32).reshape(C),
        "bvh": np.ascontiguousarray(np.asarray(bv, np.float32).reshape(C)).astype(bf),
        "wg": np.ascontiguousarray(np.asarray(Wg, np.float32).reshape(2 * C)).astype(bf),
        # "nbg" now carries +bg (Sigmoid bias), name kept for dram binding
        "nbg": np.asarray(bg, np.float32).reshape(1),
    }
    in_maps = []
    for b in range(B):
        m = dict(shared)
        m["xT"] = np.ascontiguousarray(hs[b].T).astype(bf)
        m["dep"] = np.ascontiguousarray(dep[b].T).astype(bf)
        in_maps.append(m)
    return in_maps


def kernel(**inputs):
    from concourse.bass_utils import run_bass_kernel_spmd
    if "nc" not in _CACHE:
        _CACHE["nc"] = build_nc()
    nc = _CACHE["nc"]
    in_maps = _prep_in_maps(**inputs)
    res = run_bass_kernel_spmd(nc, in_maps, core_ids=list(range(NCORES)))
    out = np.stack([res.results[i]["out"] for i in range(NCORES)], axis=0)
    return out.astype(np.float32)

